# revision 1
# baseline (speedup 1.0000x reference)
"""Trainium2 Bass kernel for nn_AttentionModel (gnn_message_passing).

Distribution (8 cores):
  - Queries (M=8192) sharded into 8 contiguous chunks of 1024. idx is sorted,
    so each core's queries live in a contiguous window of sequences; the core
    receives h_grp for just that window (row-major bf16 for gathers +
    transposed bf16 for matmuls).
  - segment_sum z: sharded by group. Each core computes z rows [512d, 512d+512)
    as a dense count-matrix matmul  z_d = C_d @ tok_emb  (both bf16; max count
    is tiny so C is exact, tok bf16 rounding is well inside the error budget),
    then AllGather (bf16, Shared output).
  - Attention is block-diagonal: queries of one sequence attend to its own 64
    positions. Blocks of BS=8 sequences; per-block query slots padded to a
    uniform CAP so the SPMD program is static.
  - All matmuls run in bf16 (1 cyc/row on PE vs 4 for fp32); f32 accumulation
    in PSUM throughout.
  - Gathers are single-shot dma_gather (int16 indices, 16-partition wrap).
    The q gathers use transpose=True, which lands rows directly in k-major
    (dh, slot) layout — no PE transposes needed for the q path.
"""

import numpy as np

N_SEQ, L, DH, DX, M, G, N_TOK, N_MEM, N_TYP = 1024, 64, 256, 128, 8192, 4096, 10000, 262144, 64
NC = 8
MC = M // NC            # queries per core
GC = G // NC            # z-groups per core
NT_PAD = ((N_TOK + 511) // 512) * 512   # 10240
KT = NT_PAD // 128
KT4 = KT // 4           # 4-k-tile DMA batches
SCALE = 1.0 / np.sqrt(np.float32(DH))
NEG = -1.0e9

_cache = {}
USE_DMA_GATHER_Q = True
USE_DMA_GATHER_Z = True
# ucode SWDGE ring holds 1024 descriptors; transpose-mode gathers with
# elem_size=256 (512B rows) cost 2 descriptors per index.
QGW = 512               # idxs per transposed q-gather call
ZGW = 1024              # idxs per z-gather call (9 calls x 8.6us beats 18 x 4.6us:
                        # per-call overhead ~1.5us dominates below 1024)


def _build(W, NBLK, BS, CAP, SLOT_PAD):
    import concourse.bacc as bacc
    import concourse.bass as bass
    import concourse.mybir as mybir
    import concourse.tile as tile
    from concourse.masks import make_identity
    from bass_rust import add_dep_helper

    f32 = mybir.dt.float32
    i16 = mybir.dt.int16
    bf16 = mybir.dt.bfloat16
    f8 = mybir.dt.float8e4
    LB = BS * L                      # l-columns per block (512 for BS=8)
    NLT = LB // 128                  # l-chunks per block (4)
    NQT = SLOT_PAD // 128            # 128-slot tiles
    NCH = SLOT_PAD // 512            # 512-slot chunks
    WL = W * L
    ZGN = NBLK * LB                  # z-gather rows (9216)
    ZCH = 3                          # z-gather chunks
    ZGC = ZGN // ZCH                 # rows per z-gather chunk
    SB = 3                           # h superblock (NBLK % 3 == 0)
    LOOK = NBLK                      # ctx after all scores (PE FIFO: ctx waits on
                                     # gathers must not block later score matmuls)

    KB_CH = 2                        # kb-batches per cmat stream DMA (KT4 % KB_CH == 0)
    nc = bacc.Bacc("TRN2", target_bir_lowering=False, num_swdge_queues=1)

    hwin = nc.declare_dram_parameter("hwin", [WL, DH], bf16, isOutput=False)
    hwinT = nc.declare_dram_parameter("hwinT", [DH, WL], bf16, isOutput=False)
    # per-partition-contiguous tilings (one big descriptor per partition line)
    tokh = nc.declare_dram_parameter("tokh", [128, KT4 * 4 * DX], bf16, isOutput=False)
    cmat = nc.declare_dram_parameter("cmat", [128, KT4 * 4 * GC], f8, isOutput=False)
    wqT = nc.declare_dram_parameter("wqT", [DH, 2 * DH], f32, isOutput=False)
    wkT = nc.declare_dram_parameter("wkT", [DH, DH], f32, isOutput=False)
    bq = nc.declare_dram_parameter("bq", [128, 2], f32, isOutput=False)
    wrel = nc.declare_dram_parameter("wrel", [2 * DH + DX, N_TYP], bf16, isOutput=False)
    brel = nc.declare_dram_parameter("brel", [N_TYP, 1], f32, isOutput=False)
    qsi = nc.declare_dram_parameter("qsi", [128, SLOT_PAD // 16], i16, isOutput=False)
    qdi = nc.declare_dram_parameter("qdi", [128, SLOT_PAD // 16], i16, isOutput=False)
    zgi = nc.declare_dram_parameter("zgi", [128, ZGN // 16], i16, isOutput=False)
    i32 = mybir.dt.int32
    qsi32 = nc.declare_dram_parameter("qsi32", [128, SLOT_PAD // 128], i32, isOutput=False)
    qdi32 = nc.declare_dram_parameter("qdi32", [128, SLOT_PAD // 128], i32, isOutput=False)
    zgi32 = nc.declare_dram_parameter("zgi32", [128, ZGN // 128], i32, isOutput=False)
    ohm = nc.declare_dram_parameter("ohm", [8, NBLK * CAP], bf16, isOutput=False)
    wmm = nc.declare_dram_parameter("wmm", [8, NBLK * LB], f8, isOutput=False)
    logitT = nc.declare_dram_parameter("logitT", [N_TYP, SLOT_PAD], f32, isOutput=True)

    z_my = nc.dram_tensor("z_my", [GC, DX], bf16)
    z_all = nc.dram_tensor("z_all", [G, DX], bf16, addr_space="Shared")

    with tile.TileContext(nc) as tc:
        with (
            tc.tile_pool(name="const", bufs=1) as const,
            tc.tile_pool(name="persist", bufs=1) as persist,
            tc.tile_pool(name="zstream", bufs=6) as zstream,
            tc.tile_pool(name="blk", bufs=4) as blk,
            tc.tile_pool(name="soft", bufs=3) as soft,
        ):
            # dummy gather: forces the Q7 SWDGE ucode LOAD_LIB to start at
            # t~=1us instead of when the first real gather issues (~19us load)
            warm_idx = const.tile([128, 8], i16, tag="warmidx")
            nc.vector.memset(warm_idx[:], 0)
            warm_out = const.tile([128, 1, DH], bf16, tag="warmout")
            nc.gpsimd.dma_gather(
                out_ap=warm_out[:], in_ap=hwin.ap(), idxs_ap=warm_idx[:],
                num_idxs=128, num_idxs_reg=128, elem_size=DH, transpose=False,
            )
            ident0 = const.tile([128, 128], f32)
            make_identity(nc, ident0[:])
            # DVE-homed bf16 identity: PE transposes depend on one engine sem.
            ident = const.tile([128, 128], bf16, tag="identW")
            nc.vector.tensor_copy(ident[:], ident0[:])

            # ---- weights / small inputs ----
            wqT_sb = [persist.tile([128, 2 * DH], f32, tag=f"wqT{j}", name=f"wqT{j}") for j in range(2)]
            for j in range(2):
                nc.scalar.dma_start(wqT_sb[j][:], wqT[j * 128:(j + 1) * 128, :])
            wkT_sb = [persist.tile([128, DH], f32, tag=f"wkT{j}", name=f"wkT{j}") for j in range(2)]
            for j in range(2):
                nc.scalar.dma_start(wkT_sb[j][:], wkT[j * 128:(j + 1) * 128, :])
            bq_sb = persist.tile([128, 2], f32, tag="bq")
            nc.scalar.dma_start(bq_sb[:], bq[:])
            wrel_sb = [persist.tile([128, N_TYP], bf16, tag=f"wrel{k}", name=f"wrel{k}") for k in range(5)]
            for k in range(5):
                nc.scalar.dma_start(wrel_sb[k][:], wrel[k * 128:(k + 1) * 128, :])
            brel_sb = persist.tile([N_TYP, 1], f32, tag="brel")
            nc.scalar.dma_start(brel_sb[:], brel[:])
            qsi_sb = persist.tile([128, SLOT_PAD // 16], i16, tag="qsi")
            nc.scalar.dma_start(qsi_sb[:], qsi[:])
            qdi_sb = persist.tile([128, SLOT_PAD // 16], i16, tag="qdi")
            nc.scalar.dma_start(qdi_sb[:], qdi[:])
            zgi_sb = persist.tile([128, ZGN // 16], i16, tag="zgi")
            nc.scalar.dma_start(zgi_sb[:], zgi[:])
            ohm_sb = persist.tile([8, NBLK * CAP], bf16, tag="ohm")
            nc.scalar.dma_start(ohm_sb[:], ohm[:])
            wmm_sb = persist.tile([8, NBLK * LB], f8, tag="wmm")
            nc.scalar.dma_start(wmm_sb[:], wmm[:])
            if not USE_DMA_GATHER_Q:
                qsi32_sb = persist.tile([128, SLOT_PAD // 128], i32, tag="qsi32")
                nc.sync.dma_start(qsi32_sb[:], qsi32[:])
                qdi32_sb = persist.tile([128, SLOT_PAD // 128], i32, tag="qdi32")
                nc.sync.dma_start(qdi32_sb[:], qdi32[:])
            if not USE_DMA_GATHER_Z:
                zgi32_sb = persist.tile([128, ZGN // 128], i32, tag="zgi32")
                nc.sync.dma_start(zgi32_sb[:], zgi32[:])

            # Wqk = Wq @ Wk^T (f32 on PE, stored bf16); bqk = Wk^T^T @ bq (f32)
            wqk_sb = [persist.tile([128, DH], bf16, tag=f"wqk{a}", name=f"wqk{a}") for a in range(4)]
            bqk_sb = [persist.tile([128, 1], f32, tag=f"bqk{c}", name=f"bqk{c}") for c in range(2)]
            with tc.tile_pool(name="pw", bufs=2, space="PSUM") as pw:
                for a in range(4):
                    pwt = pw.tile([128, DH], f32, tag="wqkps")
                    for b in range(2):
                        nc.tensor.matmul(pwt[:], lhsT=wqT_sb[b][:, a * 128:(a + 1) * 128],
                                         rhs=wkT_sb[b][:], start=(b == 0), stop=(b == 1))
                    nc.vector.tensor_copy(wqk_sb[a][:], pwt[:])
                for c in range(2):
                    pb = pw.tile([128, 1], f32, tag="bqkps")
                    for b in range(2):
                        nc.tensor.matmul(pb[:], lhsT=wkT_sb[b][:, c * 128:(c + 1) * 128],
                                         rhs=bq_sb[:, b:b + 1],
                                         start=(b == 0), stop=(b == 1))
                    nc.vector.tensor_copy(bqk_sb[c][:], pb[:])

            # front PSUM pools (Z + QK + LQ coexist): 1+1+2+2 = 6 banks
            zps_cm = tc.tile_pool(name="zps", bufs=1, space="PSUM"); zps = zps_cm.__enter__()
            ztps_cm = tc.tile_pool(name="ztps", bufs=1, space="PSUM"); ztps = ztps_cm.__enter__()
            qkps_cm = tc.tile_pool(name="qkps", bufs=2, space="PSUM"); qkps = qkps_cm.__enter__()
            lqps_cm = tc.tile_pool(name="lqps", bufs=2, space="PSUM"); lqps = lqps_cm.__enter__()

            # ---- phase Z: z_d = C_d @ tok_emb (bf16), transpose, AllGather ----
            zdT = persist.tile([DX, GC], bf16, tag="zdT")
            zrow = persist.tile([128, GC // 128 * DX], bf16, tag="zrow")
            zpsum = zps.tile([DX, GC], f32)
            # tok: one DMA into a persistent tile; cmat: KB_CH-batch stream chunks
            tokf = persist.tile([128, KT4, 4, DX], bf16, tag="tokf")
            tokh_r = tokh.rearrange("p (h r) -> h p r", h=2)
            for hh in range(2):
                nc.sync.dma_start(
                    tokf[:, hh * (KT4 // 2):(hh + 1) * (KT4 // 2), :, :]
                    .rearrange("p a b c -> p (a b c)"), tokh_r[hh])
            cmat_r = cmat.rearrange("p (cc r) -> cc p r", cc=KT4 // KB_CH)
            last_ck = None
            for cc in range(KT4 // KB_CH):
                ck = zstream.tile([128, KB_CH, 4, GC], f8, tag="ck")
                eng = nc.sync if cc % 2 == 0 else nc.scalar
                last_ck = eng.dma_start(ck[:].rearrange("p a b c -> p (a b c)"),
                                        cmat_r[cc])
                for kk in range(KB_CH):
                    kb = cc * KB_CH + kk
                    for j in range(4):
                        nc.tensor.matmul(zpsum[:], lhsT=tokf[:, kb, j, :],
                                         rhs=ck[:, kk, j, :],
                                         start=(kb == 0 and j == 0),
                                         stop=(kb == KT4 - 1 and j == 3))
            nc.vector.tensor_copy(zdT[:], zpsum[:])
            ptz = ztps.tile([128, GC // 128, 128], bf16, tag="ztp")
            for c in range(GC // 128):
                nc.tensor.transpose(ptz[:, c, :], zdT[:, c * 128:(c + 1) * 128], ident[:])
            nc.vector.tensor_copy(zrow[:], ptz[:])
            zmy_dma = nc.sync.dma_start(
                z_my.rearrange("(c p) x -> p c x", p=128),
                zrow[:].rearrange("p (c x) -> p c x", c=GC // 128))
            ag_inst = nc.gpsimd.collective_compute(
                "AllGather", mybir.AluOpType.bypass,
                replica_groups=[list(range(NC))],
                ins=[z_my.ap().opt()], outs=[z_all.ap().opt()],
            )

            # ---- q gathers: transposed single-shot -> k-major qT tiles ----
            # qT layout: [128, 2, SLOT_PAD]; dh dim j*128+d of slot s at [d, j, s]
            # chunk-major layout: [128, n_chunks, 2(dh-half), QGW]
            NQC = SLOT_PAD // QGW
            qgT = [persist.tile([128, NQC, 2, QGW], bf16, tag=f"qgT{h}", name=f"qgT{h}")
                   for h in range(2)]
            if USE_DMA_GATHER_Q:
                for h, gidx_sb in ((0, qsi_sb), (1, qdi_sb)):
                    for qc in range(NQC):
                        qgi = nc.gpsimd.dma_gather(
                            out_ap=qgT[h][:, qc, :, :],
                            in_ap=hwin.ap(),
                            idxs_ap=gidx_sb[:, qc * (QGW // 16):(qc + 1) * (QGW // 16)],
                            num_idxs=QGW, num_idxs_reg=QGW, elem_size=DH,
                            transpose=True,
                        )
                        if h == 1 and qc == 0:
                            # keep the dst half out of the Pool FIFO until z_my
                            # lands, so the AllGather issues ahead of it
                            add_dep_helper(qgi.ins, zmy_dma.ins,
                                           reason="AG before dst q-gathers")
            else:
                with (
                    tc.tile_pool(name="qga", bufs=2) as qga,
                    tc.tile_pool(name="qtps", bufs=2, space="PSUM") as qtps,
                ):
                    for h, gidx_sb in ((0, qsi32_sb), (1, qdi32_sb)):
                        qg = qga.tile([128, NQT, DH], bf16, tag=f"qg{h}", name=f"qg{h}")
                        for c8 in range(NQT):
                            nc.gpsimd.indirect_dma_start(
                                out=qg[:, c8, :], out_offset=None, in_=hwin[:],
                                in_offset=bass.IndirectOffsetOnAxis(
                                    ap=gidx_sb[:, c8:c8 + 1], axis=0),
                            )
                        for c8 in range(NQT):
                            pt = qtps.tile([128, 2, 128], bf16, tag="qtp")
                            for j in range(2):
                                nc.tensor.transpose(pt[:, j, :],
                                                    qg[:, c8, j * 128:(j + 1) * 128],
                                                    ident[:])
                            for j in range(2):
                                nc.vector.tensor_copy(
                                    qgT[h][:, c8 // 4, j,
                                           (c8 % 4) * 128:(c8 % 4 + 1) * 128],
                                    pt[:, j, :])

            def qt_a(a, ch):
                return qgT[a // 2][:, ch, a % 2, :]

            # ---- phase QK/LQ per 512-slot chunk ----
            qkT = [persist.tile([128, SLOT_PAD], bf16, tag=f"qkT{c}", name=f"qkT{c}") for c in range(2)]
            logit_q = persist.tile([N_TYP, SLOT_PAD], f32, tag="logit_q")
            for ch in range(NCH):
                sl = slice(ch * 512, (ch + 1) * 512)
                for c in range(2):
                    pq = qkps.tile([128, 512], f32, tag="qkp")
                    for a in range(4):
                        nc.tensor.matmul(pq[:], lhsT=wqk_sb[a][:, c * 128:(c + 1) * 128],
                                         rhs=qt_a(a, ch), start=(a == 0), stop=(a == 3))
                    nc.scalar.activation(qkT[c][:, sl], pq[:],
                                         mybir.ActivationFunctionType.Identity,
                                         bias=bqk_sb[c][:, :1])
                pl = lqps.tile([N_TYP, 512], f32, tag="lqp")
                for a in range(4):
                    nc.tensor.matmul(pl[:], lhsT=wrel_sb[a][:], rhs=qt_a(a, ch),
                                     start=(a == 0), stop=(a == 3))
                nc.scalar.activation(logit_q[:, sl], pl[:],
                                     mybir.ActivationFunctionType.Identity,
                                     bias=brel_sb[:, :1])

            lqps_cm.__exit__(None, None, None)
            qkps_cm.__exit__(None, None, None)
            ztps_cm.__exit__(None, None, None)
            zps_cm.__exit__(None, None, None)

            # ---- z gathers: chunked single-shot dma_gather (after AG) ----
            zg_all = persist.tile([128, NBLK * NLT, DX], bf16, tag="zg_all")
            if USE_DMA_GATHER_Z:
                for zc in range(ZGN // ZGW):
                    nc.gpsimd.dma_gather(
                        out_ap=zg_all[:, zc * (ZGW // 128):(zc + 1) * (ZGW // 128), :],
                        in_ap=z_all.ap(),
                        idxs_ap=zgi_sb[:, zc * (ZGW // 16):(zc + 1) * (ZGW // 16)],
                        num_idxs=ZGW, num_idxs_reg=ZGW, elem_size=DX,
                        transpose=False,
                    )
            else:
                for i in range(NBLK * NLT):
                    nc.gpsimd.indirect_dma_start(
                        out=zg_all[:, i, :], out_offset=None, in_=z_all.ap(),
                        in_offset=bass.IndirectOffsetOnAxis(ap=zgi32_sb[:, i:i + 1], axis=0),
                    )

            # ---- phase S: scores/softmax/attnT (S1) + ctx (S2), interleaved ----
            ctxT = persist.tile([128, SLOT_PAD], bf16, tag="ctxT")
            if NBLK * CAP < SLOT_PAD:
                nc.vector.memset(ctxT[:, NBLK * CAP:], 0.0)
            with (
                tc.tile_pool(name="sps", bufs=2, space="PSUM") as sps,
                tc.tile_pool(name="atps", bufs=3, space="PSUM") as atps,
                tc.tile_pool(name="cps", bufs=3, space="PSUM") as cps,
            ):
                hTb = [None, None]
                am = None
                aT = {}
                for bb in range(NBLK + LOOK):
                    if bb < NBLK:
                        b = bb
                        if b % SB == 0:
                            for c in range(2):
                                hTb[c] = blk.tile([128, SB * LB], bf16, tag=f"hT{c}", name=f"hT{c}")
                                h_dma = nc.scalar.dma_start(
                                    hTb[c][:],
                                    hwinT[c * 128:(c + 1) * 128, b * LB:(b + SB) * LB])
                                if b == 0:
                                    add_dep_helper(h_dma.ins, last_ck.ins,
                                                   reason="h loads after cmat stream")
                        off = (b % SB) * LB
                        hT = [hTb[c][:, off:off + LB] for c in range(2)]

                        ps_s = sps.tile([CAP, LB], f32, tag="sps")
                        for c in range(2):
                            nc.tensor.matmul(ps_s[:], lhsT=qkT[c][:, b * CAP:b * CAP + CAP],
                                             rhs=hT[c], start=(c == 0), stop=False)
                        # mask is rank-8: one-hot(slot seq-offset) x window-mask rows
                        nc.tensor.matmul(ps_s[:], lhsT=ohm_sb[:, b * CAP:b * CAP + CAP],
                                         rhs=wmm_sb[:, b * LB:(b + 1) * LB],
                                         start=False, stop=True)
                        e = soft.tile([CAP, LB], bf16, tag="e", bufs=2)
                        den = soft.tile([CAP, 1], f32, tag="den")
                        nc.scalar.activation(e[:], ps_s[:], mybir.ActivationFunctionType.Exp,
                                             scale=float(SCALE), accum_out=den[:])
                        rec = soft.tile([CAP, 1], f32, tag="rec")
                        nc.vector.reciprocal(rec[:], den[:])
                        attn = soft.tile([CAP, LB], bf16, tag="attn")
                        nc.vector.tensor_scalar_mul(attn[:], e[:], rec[:])

                        pta = atps.tile([128, NLT, CAP], bf16, tag="atp")
                        for k in range(NLT):
                            nc.tensor.transpose(pta[:, k, :], attn[:, k * 128:(k + 1) * 128],
                                                ident[:CAP, :CAP])
                        aT[b] = soft.tile([128, NLT * CAP], bf16, tag="aT", bufs=LOOK + 2,
                                          name=f"aT{b}")
                        nc.vector.tensor_copy(aT[b][:], pta[:])
                    if bb >= LOOK:
                        b2 = bb - LOOK
                        ps_c = cps.tile([DX, CAP], f32, tag="cps")
                        for k in range(NLT):
                            nc.tensor.matmul(ps_c[:], lhsT=zg_all[:, b2 * NLT + k, :],
                                             rhs=aT[b2][:, k * CAP:(k + 1) * CAP],
                                             start=(k == 0), stop=(k == NLT - 1))
                        nc.scalar.activation(ctxT[:, b2 * CAP:b2 * CAP + CAP], ps_c[:],
                                             mybir.ActivationFunctionType.Copy)
                        del aT[b2]

            # ---- phase L: logitT = logit_q + WrelC^T @ ctxT ----
            with tc.tile_pool(name="lps", bufs=2, space="PSUM") as lps:
                for ch in range(NCH):
                    pl = lps.tile([N_TYP, 512], f32, tag="lps")
                    nc.tensor.matmul(pl[:], lhsT=wrel_sb[4][:],
                                     rhs=ctxT[:, ch * 512:(ch + 1) * 512],
                                     start=True, stop=True)
                    lg = soft.tile([N_TYP, 512], f32, tag="lg", bufs=2)
                    nc.vector.tensor_add(lg[:], pl[:], logit_q[:, ch * 512:(ch + 1) * 512])
                    nc.scalar.dma_start(logitT[:, ch * 512:(ch + 1) * 512], lg[:])

    nc.compile()
    return nc


def _wrap16(flat):
    """int16 gather-index layout: index i at [i % 16, i // 16], rows tiled to 128."""
    a = np.asarray(flat, np.int16).reshape(-1, 16).T
    return np.ascontiguousarray(np.tile(a, (8, 1)))


def _prep(mem, grp, pos2grp, h_grp, msk, idx, src, dst, typ, tok_emb, Wq, bq, Wk, bk, Wrel, brel):
    """Host-side sharding/layout. Integer index work + relayout only."""
    import ml_dtypes
    bfloat16 = ml_dtypes.bfloat16
    idx = np.asarray(idx, np.int64)
    src = np.asarray(src, np.int64)
    dst = np.asarray(dst, np.int64)
    mem = np.asarray(mem, np.int64)
    grp = np.asarray(grp, np.int64)
    pos2grp = np.asarray(pos2grp, np.int64)
    msk = np.asarray(msk)
    h_grp = np.asarray(h_grp, np.float32)
    tok_emb = np.asarray(tok_emb, np.float32)

    # ---- count matrix for segment_sum ----
    C = np.bincount(grp * N_TOK + mem, minlength=G * N_TOK).reshape(G, N_TOK).astype(np.float32)

    # ---- per-core windows ----
    starts = np.array([idx[d * MC] for d in range(NC)])
    ends = np.array([idx[(d + 1) * MC - 1] for d in range(NC)])
    BS = 8
    Wmax = int((ends - starts).max()) + 1
    W = -(-Wmax // (3 * BS)) * (3 * BS)

    maxc = 0
    for d in range(NC):
        blkid = (idx[d * MC:(d + 1) * MC] - starts[d]) // BS
        maxc = max(maxc, int(np.bincount(blkid).max()))
    if maxc > 128:
        BS = 4
        W = -(-Wmax // (3 * BS)) * (3 * BS)
        maxc = 0
        for d in range(NC):
            blkid = (idx[d * MC:(d + 1) * MC] - starts[d]) // BS
            maxc = max(maxc, int(np.bincount(blkid).max()))
        assert maxc <= 128, f"block occupancy {maxc} > 128 even at BS=4"
    CAP = -(-maxc // 32) * 32
    NBLK = W // BS
    SLOT_PAD = -(-(NBLK * CAP) // 1024) * 1024
    LB = BS * L

    tok_pad = np.vstack([tok_emb, np.zeros((NT_PAD - N_TOK, DX), np.float32)])
    # per-partition-contiguous tiling: [128, KT4*4*DX], line p holds k-rows
    # {kb*512 + j*128 + p} for all (kb, j)
    tok_hi = np.ascontiguousarray(
        tok_pad.astype(bfloat16).reshape(KT4, 4, 128, DX)
        .transpose(2, 0, 1, 3).reshape(128, KT4 * 4 * DX))
    wqT_h = np.ascontiguousarray(np.asarray(Wq, np.float32).T)
    wkT_h = np.ascontiguousarray(np.asarray(Wk, np.float32).T)
    bq_h = np.ascontiguousarray(np.asarray(bq, np.float32).reshape(2, 128).T)
    wrel_h = np.ascontiguousarray(np.asarray(Wrel, np.float32).astype(bfloat16))
    brel_h = np.asarray(brel, np.float32).reshape(N_TYP, 1)

    h_flat = np.ascontiguousarray(h_grp.reshape(N_SEQ * L, DH))
    per_core = []
    slot_maps = []
    for d in range(NC):
        n_lo = int(starts[d])
        qid = idx[d * MC:(d + 1) * MC]
        qsrc = src[d * MC:(d + 1) * MC]
        qdst = dst[d * MC:(d + 1) * MC]

        hw = np.zeros((W * L, DH), np.float32)
        n_hi = min(n_lo + W, N_SEQ)
        hw[: (n_hi - n_lo) * L] = h_flat[n_lo * L: n_hi * L]
        hw_bf = hw.astype(bfloat16)
        hwT_bf = np.ascontiguousarray(hw_bf.T)

        blkid = (qid - n_lo) // BS
        cnt = np.zeros(NBLK, np.int64)
        slot = np.zeros(MC, np.int64)
        for i in range(MC):
            b = blkid[i]
            slot[i] = b * CAP + cnt[b]
            cnt[b] += 1
        slot_maps.append(slot)

        qsi_h = np.zeros(SLOT_PAD, np.int64)
        qdi_h = np.zeros(SLOT_PAD, np.int64)
        qsi_h[slot] = (qid - n_lo) * L + qsrc
        qdi_h[slot] = (qid - n_lo) * L + qdst

        p2g_pad = np.zeros((W, L), np.int64)
        p2g_pad[: n_hi - n_lo] = pos2grp[n_lo:n_hi]

        # rank-8 mask factors: mask[s, p] = sum_o oh[o, s] * wm[o, p]
        # oh: one-hot of each real slot's seq offset (pad slots all-zero ->
        # mask 0 everywhere -> finite softmax of garbage, discarded on host)
        o = (qid - n_lo) % BS
        oh = np.zeros((8, NBLK * CAP), np.float32)
        for i in range(MC):
            oh[o[i], slot[i] // CAP * CAP + slot[i] % CAP] = 1.0
        wm = np.full((8, NBLK * LB), -240.0, np.float32)
        for b in range(NBLK):
            for oo in range(BS):
                sq = n_lo + b * BS + oo
                if sq < N_SEQ:
                    wm[oo, b * LB + oo * L: b * LB + (oo + 1) * L] = np.where(
                        msk[sq].astype(bool), 0.0, -240.0)

        per_core.append({
            "hwin": hw_bf, "hwinT": hwT_bf, "tokh": tok_hi,
            "cmat": np.ascontiguousarray(
                np.vstack([C[d * GC:(d + 1) * GC].T,
                           np.zeros((NT_PAD - N_TOK, GC), np.float32)])
                .astype(ml_dtypes.float8_e4m3).reshape(KT4, 4, 128, GC)
                .transpose(2, 0, 1, 3).reshape(128, KT4 * 4 * GC)),
            "wqT": wqT_h, "wkT": wkT_h, "bq": bq_h, "wrel": wrel_h, "brel": brel_h,
            "qsi": _wrap16(qsi_h),
            "qdi": _wrap16(qdi_h),
            "zgi": _wrap16(p2g_pad.reshape(-1)),
            "qsi32": np.ascontiguousarray(
                qsi_h.reshape(SLOT_PAD // 128, 128).T.astype(np.int32)),
            "qdi32": np.ascontiguousarray(
                qdi_h.reshape(SLOT_PAD // 128, 128).T.astype(np.int32)),
            "zgi32": np.ascontiguousarray(
                p2g_pad.reshape(NBLK * (LB // 128), 128).T.astype(np.int32)),
            "ohm": oh.astype(ml_dtypes.bfloat16),
            "wmm": wm.astype(ml_dtypes.float8_e4m3),
        })
    return per_core, slot_maps, (W, NBLK, BS, CAP, SLOT_PAD)


def kernel(**inputs) -> np.ndarray:
    from concourse.bass_utils import run_bass_kernel_spmd

    per_core, slot_maps, key = _prep(**{k: inputs[k] for k in (
        "mem", "grp", "pos2grp", "h_grp", "msk", "idx", "src", "dst", "typ",
        "tok_emb", "Wq", "bq", "Wk", "bk", "Wrel", "brel")})
    if key not in _cache:
        _cache[key] = _build(*key)
    nc = _cache[key]
    res = run_bass_kernel_spmd(nc, per_core, core_ids=list(range(NC)))
    globals()["LAST_RESULT"] = res
    globals()["LAST_EXEC_NS"] = res.exec_time_ns
    out = np.empty((M, N_TYP), np.float32)
    for d in range(NC):
        out[d * MC:(d + 1) * MC] = res.results[d]["logitT"][:, slot_maps[d]].T
    return out



# revision 8
# speedup vs baseline: 1.2689x; 1.2689x over previous
"""Trainium2 Bass kernel for nn_AttentionModel (gnn_message_passing).

Distribution (8 cores):
  - Queries (M=8192) sharded into 8 contiguous chunks of 1024. idx is sorted,
    so each core's queries live in a contiguous window of sequences; the core
    receives h_grp for just that window (row-major bf16 for gathers +
    transposed bf16 for matmuls).
  - segment_sum z: sharded by group. Each core computes z rows [512d, 512d+512)
    as a dense count-matrix matmul  z_d = C_d @ tok_emb  (both bf16; max count
    is tiny so C is exact, tok bf16 rounding is well inside the error budget),
    then AllGather (bf16, Shared output).
  - Attention is block-diagonal: queries of one sequence attend to its own 64
    positions. Blocks of BS=8 sequences; per-block query slots padded to a
    uniform CAP so the SPMD program is static.
  - l-compaction: only positions with msk=1 participate in scores/ctx (the
    reference -inf's the rest), so the l axis is compacted host-side to LV
    valid slots per sequence (LV = max valid count, rounded to 16). hwinT
    columns, the window mask, and the z-gather list shrink by L/LV.
  - All matmuls run in bf16 (1 cyc/row on PE vs 4 for fp32); f32 accumulation
    in PSUM throughout.
  - Gathers are single-shot dma_gather (int16 indices, 16-partition wrap).
    The q gathers use transpose=True, which lands rows directly in k-major
    (dh, slot) layout — no PE transposes needed for the q path.
  - Schedule: cmat stream gets the DMA rings first (q-gathers dep on its last
    chunk), so z_my lands ~30us and the AllGather overlaps the q-gathers; the
    Pool engine then spends its serial desc-gen budget on z-gathers only.
"""

import numpy as np

N_SEQ, L, DH, DX, M, G, N_TOK, N_MEM, N_TYP = 1024, 64, 256, 128, 8192, 4096, 10000, 262144, 64
NC = 8
MC = M // NC            # queries per core
GC = G // NC            # z-groups per core
NT_PAD = ((N_TOK + 511) // 512) * 512   # 10240
KT = NT_PAD // 128
KT4 = KT // 4           # 4-k-tile DMA batches
SCALE = 1.0 / np.sqrt(np.float32(DH))

_cache = {}


def _build(W, NBLK, BS, CAP, SLOT_PAD, LV):
    import concourse.bacc as bacc
    import concourse.bass as bass
    import concourse.mybir as mybir
    import concourse.tile as tile
    from concourse.masks import make_identity
    from bass_rust import add_dep_helper

    f32 = mybir.dt.float32
    i16 = mybir.dt.int16
    bf16 = mybir.dt.bfloat16
    f8 = mybir.dt.float8e4
    LB = BS * LV                     # compacted l-columns per block (384)
    NLT = LB // 128                  # l-chunks per block (3)
    WL = W * L                       # rows of hwin (q gathers index full L)
    ZGN = NBLK * LB                  # z-gather rows (6912)
    SB = 3                           # h superblock (NBLK % 3 == 0)
    NSB = NBLK // SB
    LOOK = NBLK                      # ctx after all scores (PE FIFO: ctx waits on
                                     # gathers must not block later score matmuls)
    NCH = SLOT_PAD // 512            # qk/lq 512-slot chunks
    # ucode SWDGE desc ring holds 1024 descriptors per direction; transpose
    # gathers cost 2 rx-descs per index (512B rows), plain gathers 1 per side.
    QGW = 512                        # idxs per transposed q-gather call
    NQC = SLOT_PAD // QGW
    ZGW = 1024                       # max idxs per z-gather call

    KB_CH = 2                        # kb-batches per cmat stream DMA (KT4 % KB_CH == 0)
    nc = bacc.Bacc("TRN2", target_bir_lowering=False, num_swdge_queues=1)

    hwin = nc.declare_dram_parameter("hwin", [WL, DH], bf16, isOutput=False)
    hwinT = nc.declare_dram_parameter("hwinT", [DH, W * LV], bf16, isOutput=False)
    # per-partition-contiguous tilings (one big descriptor per partition line)
    tokh = nc.declare_dram_parameter("tokh", [128, KT4 * 4 * DX], bf16, isOutput=False)
    cmat = nc.declare_dram_parameter("cmat", [128, KT4 * 4 * GC], f8, isOutput=False)
    wqT = nc.declare_dram_parameter("wqT", [DH, 2 * DH], f32, isOutput=False)
    wkT = nc.declare_dram_parameter("wkT", [DH, DH], f32, isOutput=False)
    bq = nc.declare_dram_parameter("bq", [128, 2], f32, isOutput=False)
    wrel = nc.declare_dram_parameter("wrel", [2 * DH + DX, N_TYP], bf16, isOutput=False)
    brel = nc.declare_dram_parameter("brel", [N_TYP, 1], f32, isOutput=False)
    qsi = nc.declare_dram_parameter("qsi", [128, SLOT_PAD // 16], i16, isOutput=False)
    qdi = nc.declare_dram_parameter("qdi", [128, SLOT_PAD // 16], i16, isOutput=False)
    zgi = nc.declare_dram_parameter("zgi", [128, ZGN // 16], i16, isOutput=False)
    ohm = nc.declare_dram_parameter("ohm", [8, NBLK * CAP], bf16, isOutput=False)
    wmm = nc.declare_dram_parameter("wmm", [8, NBLK * LB], f8, isOutput=False)
    logitT = nc.declare_dram_parameter("logitT", [N_TYP, SLOT_PAD], f32, isOutput=True)

    z_my = nc.dram_tensor("z_my", [GC, DX], bf16)
    z_all = nc.dram_tensor("z_all", [G, DX], bf16, addr_space="Shared")

    with tile.TileContext(nc) as tc:
        with (
            tc.tile_pool(name="const", bufs=1) as const,
            tc.tile_pool(name="persist", bufs=1) as persist,
            tc.tile_pool(name="zstream", bufs=6) as zstream,
            tc.tile_pool(name="soft", bufs=3) as soft,
        ):
            # dummy gather: forces the Q7 SWDGE ucode LOAD_LIB to start at
            # t~=1us instead of when the first real gather issues (~19us load)
            warm_idx = const.tile([128, 8], i16, tag="warmidx")
            nc.vector.memset(warm_idx[:], 0)
            warm_out = const.tile([128, 1, DH], bf16, tag="warmout")
            nc.gpsimd.dma_gather(
                out_ap=warm_out[:], in_ap=hwin.ap(), idxs_ap=warm_idx[:],
                num_idxs=128, num_idxs_reg=128, elem_size=DH, transpose=False,
            )
            ident0 = const.tile([128, 128], f32)
            make_identity(nc, ident0[:])
            # DVE-homed bf16 identity: PE transposes depend on one engine sem.
            ident = const.tile([128, 128], bf16, tag="identW")
            nc.vector.tensor_copy(ident[:], ident0[:])

            # ---- weights / small inputs ----
            wqT_sb = [persist.tile([128, 2 * DH], f32, tag=f"wqT{j}", name=f"wqT{j}") for j in range(2)]
            for j in range(2):
                nc.scalar.dma_start(wqT_sb[j][:], wqT[j * 128:(j + 1) * 128, :])
            wkT_sb = [persist.tile([128, DH], f32, tag=f"wkT{j}", name=f"wkT{j}") for j in range(2)]
            for j in range(2):
                nc.scalar.dma_start(wkT_sb[j][:], wkT[j * 128:(j + 1) * 128, :])
            bq_sb = persist.tile([128, 2], f32, tag="bq")
            nc.scalar.dma_start(bq_sb[:], bq[:])
            wrel_sb = [persist.tile([128, N_TYP], bf16, tag=f"wrel{k}", name=f"wrel{k}") for k in range(5)]
            for k in range(5):
                nc.scalar.dma_start(wrel_sb[k][:], wrel[k * 128:(k + 1) * 128, :])
            brel_sb = persist.tile([N_TYP, 1], f32, tag="brel")
            nc.scalar.dma_start(brel_sb[:], brel[:])
            qsi_sb = persist.tile([128, SLOT_PAD // 16], i16, tag="qsi")
            nc.scalar.dma_start(qsi_sb[:], qsi[:])
            qdi_sb = persist.tile([128, SLOT_PAD // 16], i16, tag="qdi")
            nc.scalar.dma_start(qdi_sb[:], qdi[:])
            zgi_sb = persist.tile([128, ZGN // 16], i16, tag="zgi")
            nc.scalar.dma_start(zgi_sb[:], zgi[:])
            ohm_sb = persist.tile([8, NBLK * CAP], bf16, tag="ohm")
            nc.scalar.dma_start(ohm_sb[:], ohm[:])
            wmm_sb = persist.tile([8, NBLK * LB], f8, tag="wmm")
            nc.scalar.dma_start(wmm_sb[:], wmm[:])

            # Wqk = Wq @ Wk^T (f32 on PE, stored bf16); bqk = Wk^T^T @ bq (f32)
            wqk_sb = [persist.tile([128, DH], bf16, tag=f"wqk{a}", name=f"wqk{a}") for a in range(4)]
            bqk_sb = [persist.tile([128, 1], f32, tag=f"bqk{c}", name=f"bqk{c}") for c in range(2)]
            with tc.tile_pool(name="pw", bufs=2, space="PSUM") as pw:
                for a in range(4):
                    pwt = pw.tile([128, DH], f32, tag="wqkps")
                    for b in range(2):
                        nc.tensor.matmul(pwt[:], lhsT=wqT_sb[b][:, a * 128:(a + 1) * 128],
                                         rhs=wkT_sb[b][:], start=(b == 0), stop=(b == 1))
                    nc.vector.tensor_copy(wqk_sb[a][:], pwt[:])
                for c in range(2):
                    pb = pw.tile([128, 1], f32, tag="bqkps")
                    for b in range(2):
                        nc.tensor.matmul(pb[:], lhsT=wkT_sb[b][:, c * 128:(c + 1) * 128],
                                         rhs=bq_sb[:, b:b + 1],
                                         start=(b == 0), stop=(b == 1))
                    nc.vector.tensor_copy(bqk_sb[c][:], pb[:])

            # front PSUM pools (Z + QK + LQ coexist): 1+1+2+2 = 6 banks
            zps_cm = tc.tile_pool(name="zps", bufs=1, space="PSUM"); zps = zps_cm.__enter__()
            ztps_cm = tc.tile_pool(name="ztps", bufs=1, space="PSUM"); ztps = ztps_cm.__enter__()
            qkps_cm = tc.tile_pool(name="qkps", bufs=2, space="PSUM"); qkps = qkps_cm.__enter__()
            lqps_cm = tc.tile_pool(name="lqps", bufs=2, space="PSUM"); lqps = lqps_cm.__enter__()

            # ---- phase Z: z_d = C_d @ tok_emb (bf16), transpose, AllGather ----
            zdT = persist.tile([DX, GC], bf16, tag="zdT")
            zrow = persist.tile([128, GC // 128 * DX], bf16, tag="zrow")
            zpsum = zps.tile([DX, GC], f32)
            # tok: one DMA into a persistent tile; cmat: KB_CH-batch stream chunks
            tokf = persist.tile([128, KT4, 4, DX], bf16, tag="tokf")
            tokh_r = tokh.rearrange("p (h r) -> h p r", h=2)
            for hh in range(2):
                nc.sync.dma_start(
                    tokf[:, hh * (KT4 // 2):(hh + 1) * (KT4 // 2), :, :]
                    .rearrange("p a b c -> p (a b c)"), tokh_r[hh])
            cmat_r = cmat.rearrange("p (cc r) -> cc p r", cc=KT4 // KB_CH)
            last_ck = None
            for cc in range(KT4 // KB_CH):
                ck = zstream.tile([128, KB_CH, 4, GC], f8, tag="ck")
                eng = nc.sync if cc % 2 == 0 else nc.scalar
                last_ck = eng.dma_start(ck[:].rearrange("p a b c -> p (a b c)"),
                                        cmat_r[cc])
                for kk in range(KB_CH):
                    kb = cc * KB_CH + kk
                    for j in range(4):
                        nc.tensor.matmul(zpsum[:], lhsT=tokf[:, kb, j, :],
                                         rhs=ck[:, kk, j, :],
                                         start=(kb == 0 and j == 0),
                                         stop=(kb == KT4 - 1 and j == 3))
            nc.vector.tensor_copy(zdT[:], zpsum[:])
            ptz = ztps.tile([128, GC // 128, 128], bf16, tag="ztp")
            for c in range(GC // 128):
                nc.tensor.transpose(ptz[:, c, :], zdT[:, c * 128:(c + 1) * 128], ident[:])
            nc.vector.tensor_copy(zrow[:], ptz[:])
            zmy_dma = nc.sync.dma_start(
                z_my.rearrange("(c p) x -> p c x", p=128),
                zrow[:].rearrange("p (c x) -> p c x", c=GC // 128))
            ag_inst = nc.gpsimd.collective_compute(
                "AllGather", mybir.AluOpType.bypass,
                replica_groups=[list(range(NC))],
                ins=[z_my.ap().opt()], outs=[z_all.ap().opt()],
            )

            # ---- hT superblock preloads (sync engine, after zmy in queue) ----
            hTb = [[persist.tile([128, SB * LB], bf16, tag=f"hT{sb}_{c}",
                                 name=f"hT{sb}_{c}") for c in range(2)]
                   for sb in range(NSB)]
            for sb in range(NSB):
                for c in range(2):
                    h_dma = nc.sync.dma_start(
                        hTb[sb][c][:],
                        hwinT[c * 128:(c + 1) * 128, sb * SB * LB:(sb + 1) * SB * LB])
                    if sb == 0:
                        add_dep_helper(h_dma.ins, last_ck.ins,
                                       reason="h loads after cmat stream")

            # ---- q gathers: transposed single-shot -> k-major qT tiles ----
            # qT layout: chunk-major [128, NQC, 2, QGW]; dh dim j*128+d of
            # slot qc*QGW+s at [d, qc, j, s]
            qgT = [persist.tile([128, NQC, 2, QGW], bf16, tag=f"qgT{h}", name=f"qgT{h}")
                   for h in range(2)]
            for h, gidx_sb in ((0, qsi_sb), (1, qdi_sb)):
                for qc in range(NQC):
                    qgi = nc.gpsimd.dma_gather(
                        out_ap=qgT[h][:, qc, :, :],
                        in_ap=hwin.ap(),
                        idxs_ap=gidx_sb[:, qc * (QGW // 16):(qc + 1) * (QGW // 16)],
                        num_idxs=QGW, num_idxs_reg=QGW, elem_size=DH,
                        transpose=True,
                    )
                    if h == 0 and qc == 0:
                        # keep gather ring traffic off the cmat stream
                        add_dep_helper(qgi.ins, last_ck.ins,
                                       reason="q gathers after cmat stream")

            def qt_a(a, ch):
                return qgT[a // 2][:, ch, a % 2, :]

            # ---- phase QK/LQ per 512-slot chunk ----
            qkT = [persist.tile([128, SLOT_PAD], bf16, tag=f"qkT{c}", name=f"qkT{c}") for c in range(2)]
            logit_q = persist.tile([N_TYP, SLOT_PAD], f32, tag="logit_q")
            for ch in range(NCH):
                sl = slice(ch * 512, (ch + 1) * 512)
                for c in range(2):
                    pq = qkps.tile([128, 512], f32, tag="qkp")
                    for a in range(4):
                        nc.tensor.matmul(pq[:], lhsT=wqk_sb[a][:, c * 128:(c + 1) * 128],
                                         rhs=qt_a(a, ch), start=(a == 0), stop=(a == 3))
                    nc.scalar.activation(qkT[c][:, sl], pq[:],
                                         mybir.ActivationFunctionType.Identity,
                                         bias=bqk_sb[c][:, :1])
                pl = lqps.tile([N_TYP, 512], f32, tag="lqp")
                for a in range(4):
                    nc.tensor.matmul(pl[:], lhsT=wrel_sb[a][:], rhs=qt_a(a, ch),
                                     start=(a == 0), stop=(a == 3))
                nc.scalar.activation(logit_q[:, sl], pl[:],
                                     mybir.ActivationFunctionType.Identity,
                                     bias=brel_sb[:, :1])

            lqps_cm.__exit__(None, None, None)
            qkps_cm.__exit__(None, None, None)
            ztps_cm.__exit__(None, None, None)
            zps_cm.__exit__(None, None, None)

            # ---- z gathers: chunked single-shot dma_gather (after AG) ----
            zg_all = persist.tile([128, NBLK * NLT, DX], bf16, tag="zg_all")
            zoff = 0
            while zoff < ZGN:
                cnt = min(ZGW, ZGN - zoff)
                nc.gpsimd.dma_gather(
                    out_ap=zg_all[:, zoff // 128:(zoff + cnt) // 128, :],
                    in_ap=z_all.ap(),
                    idxs_ap=zgi_sb[:, zoff // 16:(zoff + cnt) // 16],
                    num_idxs=cnt, num_idxs_reg=cnt, elem_size=DX,
                    transpose=False,
                )
                zoff += cnt

            # ---- phase S: scores/softmax/attnT (S1) + ctx (S2), interleaved ----
            ctxT = persist.tile([128, SLOT_PAD], bf16, tag="ctxT")
            if NBLK * CAP < SLOT_PAD:
                nc.vector.memset(ctxT[:, NBLK * CAP:], 0.0)
            with (
                tc.tile_pool(name="sps", bufs=2, space="PSUM") as sps,
                tc.tile_pool(name="atps", bufs=3, space="PSUM") as atps,
                tc.tile_pool(name="cps", bufs=3, space="PSUM") as cps,
            ):
                am = None
                aT = {}
                for bb in range(NBLK + LOOK):
                    if bb < NBLK:
                        b = bb
                        hT = [hTb[b // SB][c][:, (b % SB) * LB:(b % SB + 1) * LB]
                              for c in range(2)]

                        ps_s = sps.tile([CAP, LB], f32, tag="sps")
                        for c in range(2):
                            nc.tensor.matmul(ps_s[:], lhsT=qkT[c][:, b * CAP:b * CAP + CAP],
                                             rhs=hT[c], start=(c == 0), stop=False)
                        # mask is rank-8: one-hot(slot seq-offset) x window-mask rows
                        nc.tensor.matmul(ps_s[:], lhsT=ohm_sb[:, b * CAP:b * CAP + CAP],
                                         rhs=wmm_sb[:, b * LB:(b + 1) * LB],
                                         start=False, stop=True)
                        e = soft.tile([CAP, LB], bf16, tag="e", bufs=2)
                        den = soft.tile([CAP, 1], f32, tag="den")
                        nc.scalar.activation(e[:], ps_s[:], mybir.ActivationFunctionType.Exp,
                                             scale=float(SCALE), accum_out=den[:])
                        rec = soft.tile([CAP, 1], f32, tag="rec")
                        nc.vector.reciprocal(rec[:], den[:])
                        attn = soft.tile([CAP, LB], bf16, tag="attn")
                        nc.vector.tensor_scalar_mul(attn[:], e[:], rec[:])

                        pta = atps.tile([128, NLT, CAP], bf16, tag="atp")
                        for k in range(NLT):
                            nc.tensor.transpose(pta[:, k, :], attn[:, k * 128:(k + 1) * 128],
                                                ident[:CAP, :CAP])
                        aT[b] = soft.tile([128, NLT * CAP], bf16, tag="aT", bufs=LOOK + 2,
                                          name=f"aT{b}")
                        nc.vector.tensor_copy(aT[b][:], pta[:])
                    if bb >= LOOK:
                        b2 = bb - LOOK
                        ps_c = cps.tile([DX, CAP], f32, tag="cps")
                        for k in range(NLT):
                            nc.tensor.matmul(ps_c[:], lhsT=zg_all[:, b2 * NLT + k, :],
                                             rhs=aT[b2][:, k * CAP:(k + 1) * CAP],
                                             start=(k == 0), stop=(k == NLT - 1))
                        nc.scalar.activation(ctxT[:, b2 * CAP:b2 * CAP + CAP], ps_c[:],
                                             mybir.ActivationFunctionType.Copy)
                        del aT[b2]

            # ---- phase L: logitT = logit_q + WrelC^T @ ctxT ----
            with tc.tile_pool(name="lps", bufs=2, space="PSUM") as lps:
                for ch in range(NCH):
                    w = min(512, SLOT_PAD - ch * 512)
                    sl = slice(ch * 512, ch * 512 + w)
                    pl = lps.tile([N_TYP, 512], f32, tag="lps")
                    nc.tensor.matmul(pl[:, :w], lhsT=wrel_sb[4][:],
                                     rhs=ctxT[:, sl],
                                     start=True, stop=True)
                    lg = soft.tile([N_TYP, 512], f32, tag="lg", bufs=2)
                    nc.vector.tensor_add(lg[:, :w], pl[:, :w], logit_q[:, sl])
                    nc.scalar.dma_start(logitT[:, sl], lg[:, :w])

    nc.compile()
    return nc


def _wrap16(flat):
    """int16 gather-index layout: index i at [i % 16, i // 16], rows tiled to 128."""
    a = np.asarray(flat, np.int16).reshape(-1, 16).T
    return np.ascontiguousarray(np.tile(a, (8, 1)))


def _prep(mem, grp, pos2grp, h_grp, msk, idx, src, dst, typ, tok_emb, Wq, bq, Wk, bk, Wrel, brel):
    """Host-side sharding/layout. Integer index work + relayout only."""
    import ml_dtypes
    bfloat16 = ml_dtypes.bfloat16
    idx = np.asarray(idx, np.int64)
    src = np.asarray(src, np.int64)
    dst = np.asarray(dst, np.int64)
    mem = np.asarray(mem, np.int64)
    grp = np.asarray(grp, np.int64)
    pos2grp = np.asarray(pos2grp, np.int64)
    msk = np.asarray(msk)
    h_grp = np.asarray(h_grp, np.float32)
    tok_emb = np.asarray(tok_emb, np.float32)

    # ---- count matrix for segment_sum ----
    C = np.bincount(grp * N_TOK + mem, minlength=G * N_TOK).reshape(G, N_TOK).astype(np.float32)

    # ---- per-core windows ----
    starts = np.array([idx[d * MC] for d in range(NC)])
    ends = np.array([idx[(d + 1) * MC - 1] for d in range(NC)])
    BS = 8
    Wmax = int((ends - starts).max()) + 1
    W = -(-Wmax // (3 * BS)) * (3 * BS)

    maxc = 0
    for d in range(NC):
        blkid = (idx[d * MC:(d + 1) * MC] - starts[d]) // BS
        maxc = max(maxc, int(np.bincount(blkid).max()))
    if maxc > 128:
        BS = 4
        W = -(-Wmax // (3 * BS)) * (3 * BS)
        maxc = 0
        for d in range(NC):
            blkid = (idx[d * MC:(d + 1) * MC] - starts[d]) // BS
            maxc = max(maxc, int(np.bincount(blkid).max()))
        assert maxc <= 128, f"block occupancy {maxc} > 128 even at BS=4"
    CAP = -(-maxc // 32) * 32
    NBLK = W // BS
    SLOT_PAD = -(-(NBLK * CAP) // 512) * 512
    # l-compaction: LV = max valid positions per sequence, 16-aligned so
    # BS*LV is a multiple of 128 (BS=8).
    if BS == 8:
        lv_max = int(msk.sum(axis=1).max())
        LV = min(L, -(-lv_max // 16) * 16)
    else:
        LV = L
    LB = BS * LV

    # per-seq valid position lists, padded with position 0 (masked out)
    vcnt = msk.sum(axis=1).astype(np.int64)
    vpos = np.zeros((N_SEQ, LV), np.int64)
    for s in range(N_SEQ):
        v = np.flatnonzero(msk[s])[:LV]
        vpos[s, :len(v)] = v

    tok_pad = np.vstack([tok_emb, np.zeros((NT_PAD - N_TOK, DX), np.float32)])
    # per-partition-contiguous tiling: [128, KT4*4*DX], line p holds k-rows
    # {kb*512 + j*128 + p} for all (kb, j)
    tok_hi = np.ascontiguousarray(
        tok_pad.astype(bfloat16).reshape(KT4, 4, 128, DX)
        .transpose(2, 0, 1, 3).reshape(128, KT4 * 4 * DX))
    wqT_h = np.ascontiguousarray(np.asarray(Wq, np.float32).T)
    wkT_h = np.ascontiguousarray(np.asarray(Wk, np.float32).T)
    bq_h = np.ascontiguousarray(np.asarray(bq, np.float32).reshape(2, 128).T)
    wrel_h = np.ascontiguousarray(np.asarray(Wrel, np.float32).astype(bfloat16))
    brel_h = np.asarray(brel, np.float32).reshape(N_TYP, 1)

    h_flat = np.ascontiguousarray(h_grp.reshape(N_SEQ * L, DH))
    per_core = []
    slot_maps = []
    for d in range(NC):
        n_lo = int(starts[d])
        qid = idx[d * MC:(d + 1) * MC]
        qsrc = src[d * MC:(d + 1) * MC]
        qdst = dst[d * MC:(d + 1) * MC]

        hw = np.zeros((W * L, DH), np.float32)
        n_hi = min(n_lo + W, N_SEQ)
        hw[: (n_hi - n_lo) * L] = h_flat[n_lo * L: n_hi * L]
        hw_bf = hw.astype(bfloat16)

        # compacted transposed window: column (s_local*LV + j) = h[s, vpos[s, j]]
        hwc = np.zeros((W * LV, DH), np.float32)
        srows = np.arange(n_lo, n_hi)
        sel = (srows[:, None] * L + vpos[srows]).reshape(-1)
        hwc[: (n_hi - n_lo) * LV] = h_flat[sel]
        # zero out per-seq padding columns (j >= vcnt[s])
        padm = (np.arange(LV)[None, :] >= vcnt[srows][:, None]).reshape(-1)
        hwc[: (n_hi - n_lo) * LV][padm] = 0.0
        hwcT_bf = np.ascontiguousarray(hwc.astype(bfloat16).T)

        blkid = (qid - n_lo) // BS
        cnt = np.zeros(NBLK, np.int64)
        slot = np.zeros(MC, np.int64)
        for i in range(MC):
            b = blkid[i]
            slot[i] = b * CAP + cnt[b]
            cnt[b] += 1
        slot_maps.append(slot)

        qsi_h = np.zeros(SLOT_PAD, np.int64)
        qdi_h = np.zeros(SLOT_PAD, np.int64)
        qsi_h[slot] = (qid - n_lo) * L + qsrc
        qdi_h[slot] = (qid - n_lo) * L + qdst

        # compacted pos->group: row s_local, LV entries (pad -> p2g[s, 0])
        p2g_pad = np.zeros((W, LV), np.int64)
        p2g_pad[: n_hi - n_lo] = pos2grp[srows[:, None], vpos[srows]]

        # rank-8 mask factors: mask[s, p] = sum_o oh[o, s] * wm[o, p]
        # oh: one-hot of each real slot's seq offset (pad slots all-zero ->
        # mask 0 everywhere -> finite softmax of garbage, discarded on host)
        o = (qid - n_lo) % BS
        oh = np.zeros((8, NBLK * CAP), np.float32)
        for i in range(MC):
            oh[o[i], slot[i]] = 1.0
        wm = np.full((8, NBLK * LB), -240.0, np.float32)
        for b in range(NBLK):
            for oo in range(BS):
                sq = n_lo + b * BS + oo
                if sq < N_SEQ:
                    wm[oo, b * LB + oo * LV: b * LB + oo * LV + int(vcnt[sq])] = 0.0

        per_core.append({
            "hwin": hw_bf, "hwinT": hwcT_bf, "tokh": tok_hi,
            "cmat": np.ascontiguousarray(
                np.vstack([C[d * GC:(d + 1) * GC].T,
                           np.zeros((NT_PAD - N_TOK, GC), np.float32)])
                .astype(ml_dtypes.float8_e4m3).reshape(KT4, 4, 128, GC)
                .transpose(2, 0, 1, 3).reshape(128, KT4 * 4 * GC)),
            "wqT": wqT_h, "wkT": wkT_h, "bq": bq_h, "wrel": wrel_h, "brel": brel_h,
            "qsi": _wrap16(qsi_h),
            "qdi": _wrap16(qdi_h),
            "zgi": _wrap16(p2g_pad.reshape(-1)),
            "ohm": oh.astype(ml_dtypes.bfloat16),
            "wmm": wm.astype(ml_dtypes.float8_e4m3),
        })
    return per_core, slot_maps, (W, NBLK, BS, CAP, SLOT_PAD, LV)


def kernel(**inputs) -> np.ndarray:
    from concourse.bass_utils import run_bass_kernel_spmd

    per_core, slot_maps, key = _prep(**{k: inputs[k] for k in (
        "mem", "grp", "pos2grp", "h_grp", "msk", "idx", "src", "dst", "typ",
        "tok_emb", "Wq", "bq", "Wk", "bk", "Wrel", "brel")})
    if key not in _cache:
        _cache[key] = _build(*key)
    nc = _cache[key]
    res = run_bass_kernel_spmd(nc, per_core, core_ids=list(range(NC)))
    globals()["LAST_RESULT"] = res
    globals()["LAST_EXEC_NS"] = res.exec_time_ns
    out = np.empty((M, N_TYP), np.float32)
    for d in range(NC):
        out[d * MC:(d + 1) * MC] = res.results[d]["logitT"][:, slot_maps[d]].T
    return out


# revision 26
# speedup vs baseline: 1.2745x; 1.0044x over previous
"""Trainium2 Bass kernel for nn_AttentionModel (gnn_message_passing).

Distribution (8 cores):
  - Queries (M=8192) sharded into 8 contiguous chunks of 1024. idx is sorted,
    so each core's queries live in a contiguous window of sequences; the core
    receives h_grp for just that window (row-major bf16 for gathers +
    transposed bf16 for matmuls).
  - segment_sum z: sharded by group. Each core computes z rows [512d, 512d+512)
    as a dense count-matrix matmul  z_d = C_d @ tok_emb  (both bf16; max count
    is tiny so C is exact, tok bf16 rounding is well inside the error budget),
    then AllGather (bf16, Shared output).
  - Attention is block-diagonal: queries of one sequence attend to its own 64
    positions. Blocks of BS=8 sequences; per-block query slots padded to a
    uniform CAP so the SPMD program is static.
  - l-compaction: only positions with msk=1 participate in scores/ctx (the
    reference -inf's the rest), so the l axis is compacted host-side to LV
    valid slots per sequence (LV = max valid count, rounded to 16). hwinT
    columns, the window mask, and the z-gather list shrink by L/LV.
  - All matmuls run in bf16 (1 cyc/row on PE vs 4 for fp32); f32 accumulation
    in PSUM throughout.
  - Gathers are single-shot dma_gather (int16 indices, 16-partition wrap).
    The q gathers use transpose=True, which lands rows directly in k-major
    (dh, slot) layout — no PE transposes needed for the q path.
  - Schedule: cmat stream gets the DMA rings first (q-gathers dep on its last
    chunk), so z_my lands ~30us and the AllGather overlaps the q-gathers; the
    Pool engine then spends its serial desc-gen budget on z-gathers only.
"""

import numpy as np

N_SEQ, L, DH, DX, M, G, N_TOK, N_MEM, N_TYP = 1024, 64, 256, 128, 8192, 4096, 10000, 262144, 64
NC = 8
MC = M // NC            # queries per core
GC = G // NC            # z-groups per core
NT_PAD = ((N_TOK + 511) // 512) * 512   # 10240
KT = NT_PAD // 128
KT4 = KT // 4           # 4-k-tile DMA batches
SCALE = 1.0 / np.sqrt(np.float32(DH))

_cache = {}


def _build(W, NBLK, BS, CAP, SLOT_PAD, LV):
    import concourse.bacc as bacc
    import concourse.bass as bass
    import concourse.mybir as mybir
    import concourse.tile as tile
    from concourse.masks import make_identity
    from bass_rust import add_dep_helper

    f32 = mybir.dt.float32
    i16 = mybir.dt.int16
    bf16 = mybir.dt.bfloat16
    f8 = mybir.dt.float8e4
    LB = BS * LV                     # compacted l-columns per block (384)
    NLT = LB // 128                  # l-chunks per block (3)
    WL = W * L                       # rows of hwin (q gathers index full L)
    ZGN = NBLK * LB                  # z-gather rows (6912)
    SB = 3                           # h superblock (NBLK % 3 == 0)
    NSB = NBLK // SB
    LOOK = NBLK                      # ctx after all scores (PE FIFO: ctx waits on
                                     # gathers must not block later score matmuls)
    NCH = SLOT_PAD // 512            # qk/lq 512-slot chunks
    # ucode SWDGE desc ring holds 1024 descriptors per direction; transpose
    # gathers cost 2 rx-descs per index (512B rows), plain gathers 1 per side.
    QGW = 512                        # idxs per transposed q-gather call
    NQC = SLOT_PAD // QGW
    ZGW = 1024                       # max idxs per z-gather call
    NI = SLOT_PAD // 16 * 2 + ZGN // 16   # int16 index blob columns

    KB_CH = 5                        # kb-batches per cmat stream DMA (4 fat chunks:
                                     # DMA rings cost ~155ns/partition-line, so
                                     # fewer+fatter DMAs everywhere on the hot path)
    nc = bacc.Bacc("TRN2", target_bir_lowering=False, num_swdge_queues=1)

    hwin = nc.declare_dram_parameter("hwin", [WL, DH], bf16, isOutput=False)
    hwinT = nc.declare_dram_parameter("hwinT", [DH, W * LV], bf16, isOutput=False)
    # per-partition-contiguous tilings (one big descriptor per partition line)
    tokh = nc.declare_dram_parameter("tokh", [128, KT4 * 4 * DX], bf16, isOutput=False)
    cmat = nc.declare_dram_parameter("cmat", [128, KT4 * 4 * GC], f8, isOutput=False)
    # wb16: per-row [wqT[p], wqT[128+p], wkT[p], wkT[128+p], wrel 5x64]
    wb16 = nc.declare_dram_parameter("wb16", [128, 1858], bf16, isOutput=False)
    # wb32: [bq (2 cols), brel (rows 0:64 of col 2)]
    wb32 = nc.declare_dram_parameter("wb32", [128, 3], f32, isOutput=False)
    ib16 = nc.declare_dram_parameter("ib16", [128, NI], i16, isOutput=False)
    ohm = nc.declare_dram_parameter("ohm", [8, NBLK * CAP], bf16, isOutput=False)
    wmm = nc.declare_dram_parameter("wmm", [8, NBLK * LB], f8, isOutput=False)
    logitT = nc.declare_dram_parameter("logitT", [N_TYP, SLOT_PAD], f32, isOutput=True)

    # z_my flat [128, GC//128*DX]: local group gl lives at row gl%128, chunk
    # gl//128 (the host remaps zgi accordingly; saves a 4x descriptor fan-out)
    z_my = nc.dram_tensor("z_my", [128, GC // 128 * DX], bf16)
    z_all = nc.dram_tensor("z_all", [G, DX], bf16, addr_space="Shared")

    with tile.TileContext(nc) as tc:
        with (
            tc.tile_pool(name="const", bufs=1) as const,
            tc.tile_pool(name="persist", bufs=1) as persist,
            tc.tile_pool(name="zstream", bufs=4) as zstream,
            tc.tile_pool(name="soft", bufs=3) as soft,
        ):
            # dummy gather: forces the Q7 SWDGE ucode LOAD_LIB to start at
            # t~=1us instead of when the first real gather issues (~19us load)
            warm_idx = const.tile([128, 8], i16, tag="warmidx")
            nc.vector.memset(warm_idx[:], 0)
            warm_out = const.tile([128, 1, DH], bf16, tag="warmout")
            nc.gpsimd.dma_gather(
                out_ap=warm_out[:], in_ap=hwin.ap(), idxs_ap=warm_idx[:],
                num_idxs=128, num_idxs_reg=128, elem_size=DH, transpose=False,
            )
            ident0 = const.tile([128, 128], f32)
            make_identity(nc, ident0[:])
            # DVE-homed bf16 identity: PE transposes depend on one engine sem.
            ident = const.tile([128, 128], bf16, tag="identW")
            nc.vector.tensor_copy(ident[:], ident0[:])

            # ---- weights / small inputs (batched into few fat DMAs) ----
            wb16_sb = persist.tile([128, 1858], bf16, tag="wb16")
            nc.scalar.dma_start(wb16_sb[:], wb16[:])
            wb32_sb = persist.tile([128, 3], f32, tag="wb32")
            nc.scalar.dma_start(wb32_sb[:], wb32[:])
            ib16_sb = persist.tile([128, NI], i16, tag="ib16")
            nc.scalar.dma_start(ib16_sb[:], ib16[:])
            ohm_sb = persist.tile([8, NBLK * CAP], bf16, tag="ohm")
            nc.scalar.dma_start(ohm_sb[:], ohm[:])
            wmm_sb = persist.tile([8, NBLK * LB], f8, tag="wmm")
            nc.scalar.dma_start(wmm_sb[:], wmm[:])
            def wqT_s(b, lo, hi):
                return wb16_sb[:, b * 512 + lo:b * 512 + hi]

            def wkT_s(b, lo, hi):
                return wb16_sb[:, 1024 + b * 256 + lo:1024 + b * 256 + hi]

            def wrel_s(k):
                return wb16_sb[:, 1536 + k * 64:1536 + (k + 1) * 64]

            # front PSUM pools (Z + wqk + QK + LQ coexist): 1+1+2+2+2 = 8 banks
            zps_cm = tc.tile_pool(name="zps", bufs=1, space="PSUM"); zps = zps_cm.__enter__()
            ztps_cm = tc.tile_pool(name="ztps", bufs=1, space="PSUM"); ztps = ztps_cm.__enter__()
            qkps_cm = tc.tile_pool(name="qkps", bufs=2, space="PSUM"); qkps = qkps_cm.__enter__()
            lqps_cm = tc.tile_pool(name="lqps", bufs=2, space="PSUM"); lqps = lqps_cm.__enter__()
            pw_cm = tc.tile_pool(name="pw", bufs=1, space="PSUM"); pw = pw_cm.__enter__()

            # ---- phase Z: z_d = C_d @ tok_emb (bf16), transpose, AllGather ----
            # (Z runs FIRST on the PE; wqk after, while the AllGather flies)
            zdT = persist.tile([DX, GC], bf16, tag="zdT")
            zrow = persist.tile([128, GC // 128 * DX], bf16, tag="zrow")
            zpsum = zps.tile([DX, GC], f32)
            # tok halves + 4 fat cmat chunks interleaved on both HWDGE queues
            tokf = persist.tile([128, KT4, 4, DX], bf16, tag="tokf")
            tokh_r = tokh.rearrange("p (h r) -> h p r", h=2)
            cmat_r = cmat.rearrange("p (cc r) -> cc p r", cc=KT4 // KB_CH)
            nc.sync.dma_start(
                tokf[:, :KT4 // 2, :, :].rearrange("p a b c -> p (a b c)"), tokh_r[0])
            ck_dmas = []
            cks = []
            for cc in range(KT4 // KB_CH):
                ck = zstream.tile([128, KB_CH, 4, GC], f8, tag="ck")
                cks.append(ck)
                eng = nc.sync if cc % 2 == 0 else nc.scalar
                ck_dmas.append(eng.dma_start(
                    ck[:].rearrange("p a b c -> p (a b c)"), cmat_r[cc]))
                if cc == 0:
                    nc.sync.dma_start(
                        tokf[:, KT4 // 2:, :, :].rearrange("p a b c -> p (a b c)"),
                        tokh_r[1])
            for cc in range(KT4 // KB_CH):
                for kk in range(KB_CH):
                    kb = cc * KB_CH + kk
                    for j in range(4):
                        nc.tensor.matmul(zpsum[:], lhsT=tokf[:, kb, j, :],
                                         rhs=cks[cc][:, kk, j, :],
                                         start=(kb == 0 and j == 0),
                                         stop=(kb == KT4 - 1 and j == 3))
            nc.vector.tensor_copy(zdT[:], zpsum[:])
            ptz = ztps.tile([128, GC // 128, 128], bf16, tag="ztp")
            for c in range(GC // 128):
                nc.tensor.transpose(ptz[:, c, :], zdT[:, c * 128:(c + 1) * 128], ident[:])
            nc.vector.tensor_copy(zrow[:], ptz[:])
            zmy_dma = nc.sync.dma_start(z_my[:], zrow[:])
            ag_inst = nc.gpsimd.collective_compute(
                "AllGather", mybir.AluOpType.bypass,
                replica_groups=[list(range(NC))],
                ins=[z_my.ap().opt()], outs=[z_all.ap().opt()],
            )

            # Wqk = Wq @ Wk^T (bf16 in, f32 acc); bqk = Wk^T^T @ bq
            wqk_sb = [persist.tile([128, DH], bf16, tag=f"wqk{a}", name=f"wqk{a}") for a in range(4)]
            bqk_sb = [persist.tile([128, 1], f32, tag=f"bqk{c}", name=f"bqk{c}") for c in range(2)]
            for a in range(4):
                pwt = pw.tile([128, DH], f32, tag="wqkps")
                for b in range(2):
                    nc.tensor.matmul(pwt[:], lhsT=wqT_s(b, a * 128, (a + 1) * 128),
                                     rhs=wkT_s(b, 0, 256), start=(b == 0), stop=(b == 1))
                nc.vector.tensor_copy(wqk_sb[a][:], pwt[:])
            for c in range(2):
                pb = pw.tile([128, 1], f32, tag="bqkps")
                for b in range(2):
                    nc.tensor.matmul(pb[:], lhsT=wkT_s(b, c * 128, (c + 1) * 128),
                                     rhs=wb16_sb[:, 1856 + b:1857 + b],
                                     start=(b == 0), stop=(b == 1))
                nc.vector.tensor_copy(bqk_sb[c][:], pb[:])

            # ---- hT whole-window loads (one fat DMA per dh half) ----
            hTw = [persist.tile([128, W * LV], bf16, tag=f"hTw{c}", name=f"hTw{c}")
                   for c in range(2)]
            for c in range(2):
                h_dma = nc.scalar.dma_start(hTw[c][:], hwinT[c * 128:(c + 1) * 128, :])
                add_dep_helper(h_dma.ins, ck_dmas[3 - c].ins,
                               reason="h loads after cmat stream")

            # ---- q gathers: transposed single-shot -> k-major qT tiles ----
            # qT layout: chunk-major [128, NQC, 2, QGW]; dh dim j*128+d of
            # slot qc*QGW+s at [d, qc, j, s]
            qgT = [persist.tile([128, NQC, 2, QGW], bf16, tag=f"qgT{h}", name=f"qgT{h}")
                   for h in range(2)]
            for h in range(2):
                ib_off = h * (SLOT_PAD // 16)
                for qc in range(NQC):
                    qgi = nc.gpsimd.dma_gather(
                        out_ap=qgT[h][:, qc, :, :],
                        in_ap=hwin.ap(),
                        idxs_ap=ib16_sb[:, ib_off + qc * (QGW // 16):
                                        ib_off + (qc + 1) * (QGW // 16)],
                        num_idxs=QGW, num_idxs_reg=QGW, elem_size=DH,
                        transpose=True,
                    )
                    if h == 0 and qc == 0:
                        # keep gather ring traffic off the cmat stream
                        for ckd in ck_dmas[2:]:
                            add_dep_helper(qgi.ins, ckd.ins,
                                           reason="q gathers after cmat stream")

            def qt_a(a, ch):
                return qgT[a // 2][:, ch, a % 2, :]

            # ---- phase QK/LQ per 512-slot chunk ----
            qkT = [persist.tile([128, SLOT_PAD], bf16, tag=f"qkT{c}", name=f"qkT{c}") for c in range(2)]
            logit_q = persist.tile([N_TYP, SLOT_PAD], f32, tag="logit_q")
            for ch in range(NCH):
                sl = slice(ch * 512, (ch + 1) * 512)
                for c in range(2):
                    pq = qkps.tile([128, 512], f32, tag="qkp")
                    for a in range(4):
                        nc.tensor.matmul(pq[:], lhsT=wqk_sb[a][:, c * 128:(c + 1) * 128],
                                         rhs=qt_a(a, ch), start=(a == 0), stop=(a == 3))
                    nc.scalar.activation(qkT[c][:, sl], pq[:],
                                         mybir.ActivationFunctionType.Identity,
                                         bias=bqk_sb[c][:, :1])
                pl = lqps.tile([N_TYP, 512], f32, tag="lqp")
                for a in range(4):
                    nc.tensor.matmul(pl[:], lhsT=wrel_s(a), rhs=qt_a(a, ch),
                                     start=(a == 0), stop=(a == 3))
                nc.scalar.activation(logit_q[:, sl], pl[:],
                                     mybir.ActivationFunctionType.Identity,
                                     bias=wb32_sb[0:N_TYP, 2:3])

            pw_cm.__exit__(None, None, None)
            lqps_cm.__exit__(None, None, None)
            qkps_cm.__exit__(None, None, None)
            ztps_cm.__exit__(None, None, None)
            zps_cm.__exit__(None, None, None)

            # ---- z gathers: chunked single-shot dma_gather (after AG) ----
            zg_all = persist.tile([128, NBLK * NLT, DX], bf16, tag="zg_all")
            zoff = 0
            while zoff < ZGN:
                cnt = min(ZGW, ZGN - zoff)
                nc.gpsimd.dma_gather(
                    out_ap=zg_all[:, zoff // 128:(zoff + cnt) // 128, :],
                    in_ap=z_all.ap(),
                    idxs_ap=ib16_sb[:, SLOT_PAD // 8 + zoff // 16:
                                    SLOT_PAD // 8 + (zoff + cnt) // 16],
                    num_idxs=cnt, num_idxs_reg=cnt, elem_size=DX,
                    transpose=False,
                )
                zoff += cnt

            # ---- phase S: scores/softmax/attnT (S1) + ctx (S2), interleaved ----
            ctxT = persist.tile([128, SLOT_PAD], bf16, tag="ctxT")
            if NBLK * CAP < SLOT_PAD:
                nc.vector.memset(ctxT[:, NBLK * CAP:], 0.0)
            with (
                tc.tile_pool(name="sps", bufs=2, space="PSUM") as sps,
                tc.tile_pool(name="atps", bufs=3, space="PSUM") as atps,
                tc.tile_pool(name="cps", bufs=3, space="PSUM") as cps,
            ):
                am = None
                aT = {}
                for bb in range(NBLK + LOOK):
                    if bb < NBLK:
                        b = bb
                        hT = [hTw[c][:, b * LB:(b + 1) * LB] for c in range(2)]

                        ps_s = sps.tile([CAP, LB], f32, tag="sps")
                        for c in range(2):
                            nc.tensor.matmul(ps_s[:], lhsT=qkT[c][:, b * CAP:b * CAP + CAP],
                                             rhs=hT[c], start=(c == 0), stop=False)
                        # mask is rank-8: one-hot(slot seq-offset) x window-mask rows
                        nc.tensor.matmul(ps_s[:], lhsT=ohm_sb[:, b * CAP:b * CAP + CAP],
                                         rhs=wmm_sb[:, b * LB:(b + 1) * LB],
                                         start=False, stop=True)
                        e = soft.tile([CAP, LB], bf16, tag="e", bufs=2)
                        den = soft.tile([CAP, 1], f32, tag="den")
                        nc.scalar.activation(e[:], ps_s[:], mybir.ActivationFunctionType.Exp,
                                             scale=float(SCALE), accum_out=den[:])
                        rec = soft.tile([CAP, 1], f32, tag="rec")
                        nc.vector.reciprocal(rec[:], den[:])
                        attn = soft.tile([CAP, LB], bf16, tag="attn")
                        nc.vector.tensor_scalar_mul(attn[:], e[:], rec[:])

                        pta = atps.tile([128, NLT, CAP], bf16, tag="atp")
                        for k in range(NLT):
                            nc.tensor.transpose(pta[:, k, :], attn[:, k * 128:(k + 1) * 128],
                                                ident[:CAP, :CAP])
                        aT[b] = soft.tile([128, NLT * CAP], bf16, tag="aT", bufs=LOOK + 2,
                                          name=f"aT{b}")
                        nc.vector.tensor_copy(aT[b][:], pta[:])
                    if bb >= LOOK:
                        b2 = bb - LOOK
                        ps_c = cps.tile([DX, CAP], f32, tag="cps")
                        for k in range(NLT):
                            nc.tensor.matmul(ps_c[:], lhsT=zg_all[:, b2 * NLT + k, :],
                                             rhs=aT[b2][:, k * CAP:(k + 1) * CAP],
                                             start=(k == 0), stop=(k == NLT - 1))
                        nc.scalar.activation(ctxT[:, b2 * CAP:b2 * CAP + CAP], ps_c[:],
                                             mybir.ActivationFunctionType.Copy)
                        del aT[b2]

            # ---- phase L: logitT = logit_q + WrelC^T @ ctxT ----
            with tc.tile_pool(name="lps", bufs=2, space="PSUM") as lps:
                for ch in range(NCH):
                    w = min(512, SLOT_PAD - ch * 512)
                    sl = slice(ch * 512, ch * 512 + w)
                    pl = lps.tile([N_TYP, 512], f32, tag="lps")
                    nc.tensor.matmul(pl[:, :w], lhsT=wrel_s(4),
                                     rhs=ctxT[:, sl],
                                     start=True, stop=True)
                    lg = soft.tile([N_TYP, 512], f32, tag="lg", bufs=2)
                    nc.vector.tensor_add(lg[:, :w], pl[:, :w], logit_q[:, sl])
                    nc.scalar.dma_start(logitT[:, sl], lg[:, :w])

    nc.compile()
    return nc


def _wrap16(flat):
    """int16 gather-index layout: index i at [i % 16, i // 16], rows tiled to 128."""
    a = np.asarray(flat, np.int16).reshape(-1, 16).T
    return np.ascontiguousarray(np.tile(a, (8, 1)))


def _prep(mem, grp, pos2grp, h_grp, msk, idx, src, dst, typ, tok_emb, Wq, bq, Wk, bk, Wrel, brel):
    """Host-side sharding/layout. Integer index work + relayout only."""
    import ml_dtypes
    bfloat16 = ml_dtypes.bfloat16
    idx = np.asarray(idx, np.int64)
    src = np.asarray(src, np.int64)
    dst = np.asarray(dst, np.int64)
    mem = np.asarray(mem, np.int64)
    grp = np.asarray(grp, np.int64)
    pos2grp = np.asarray(pos2grp, np.int64)
    msk = np.asarray(msk)
    h_grp = np.asarray(h_grp, np.float32)
    tok_emb = np.asarray(tok_emb, np.float32)

    # ---- count matrix for segment_sum ----
    C = np.bincount(grp * N_TOK + mem, minlength=G * N_TOK).reshape(G, N_TOK).astype(np.float32)

    # ---- per-core windows ----
    starts = np.array([idx[d * MC] for d in range(NC)])
    ends = np.array([idx[(d + 1) * MC - 1] for d in range(NC)])
    BS = 8
    Wmax = int((ends - starts).max()) + 1
    W = -(-Wmax // (3 * BS)) * (3 * BS)

    maxc = 0
    for d in range(NC):
        blkid = (idx[d * MC:(d + 1) * MC] - starts[d]) // BS
        maxc = max(maxc, int(np.bincount(blkid).max()))
    if maxc > 128:
        BS = 4
        W = -(-Wmax // (3 * BS)) * (3 * BS)
        maxc = 0
        for d in range(NC):
            blkid = (idx[d * MC:(d + 1) * MC] - starts[d]) // BS
            maxc = max(maxc, int(np.bincount(blkid).max()))
        assert maxc <= 128, f"block occupancy {maxc} > 128 even at BS=4"
    CAP = -(-maxc // 32) * 32
    NBLK = W // BS
    SLOT_PAD = -(-(NBLK * CAP) // 512) * 512
    # l-compaction: LV = max valid positions per sequence, 16-aligned so
    # BS*LV is a multiple of 128 (BS=8).
    if BS == 8:
        lv_max = int(msk.sum(axis=1).max())
        LV = min(L, -(-lv_max // 16) * 16)
    else:
        LV = L
    LB = BS * LV

    # per-seq valid position lists, padded with position 0 (masked out)
    vcnt = msk.sum(axis=1).astype(np.int64)
    vpos = np.zeros((N_SEQ, LV), np.int64)
    for s in range(N_SEQ):
        v = np.flatnonzero(msk[s])[:LV]
        vpos[s, :len(v)] = v

    tok_pad = np.vstack([tok_emb, np.zeros((NT_PAD - N_TOK, DX), np.float32)])
    # per-partition-contiguous tiling: [128, KT4*4*DX], line p holds k-rows
    # {kb*512 + j*128 + p} for all (kb, j)
    tok_hi = np.ascontiguousarray(
        tok_pad.astype(bfloat16).reshape(KT4, 4, 128, DX)
        .transpose(2, 0, 1, 3).reshape(128, KT4 * 4 * DX))
    wqT_h = np.asarray(Wq, np.float32).T.astype(bfloat16)
    wkT_h = np.asarray(Wk, np.float32).T.astype(bfloat16)
    wrel_h = np.asarray(Wrel, np.float32).astype(bfloat16)
    wb16_h = np.ascontiguousarray(np.concatenate(
        [wqT_h[:128], wqT_h[128:], wkT_h[:128], wkT_h[128:]]
        + [wrel_h[k * 128:(k + 1) * 128] for k in range(5)], axis=1))
    bq_cols = np.asarray(bq, np.float32).reshape(2, 128).T
    wb16_h = np.ascontiguousarray(np.concatenate(
        [wb16_h, bq_cols.astype(bfloat16)], axis=1))
    wb32_h = np.zeros((128, 3), np.float32)
    wb32_h[:N_TYP, 2] = np.asarray(brel, np.float32)

    h_flat = np.ascontiguousarray(h_grp.reshape(N_SEQ * L, DH))
    per_core = []
    slot_maps = []
    for d in range(NC):
        n_lo = int(starts[d])
        qid = idx[d * MC:(d + 1) * MC]
        qsrc = src[d * MC:(d + 1) * MC]
        qdst = dst[d * MC:(d + 1) * MC]

        hw = np.zeros((W * L, DH), np.float32)
        n_hi = min(n_lo + W, N_SEQ)
        hw[: (n_hi - n_lo) * L] = h_flat[n_lo * L: n_hi * L]
        hw_bf = hw.astype(bfloat16)

        # compacted transposed window: column (s_local*LV + j) = h[s, vpos[s, j]]
        hwc = np.zeros((W * LV, DH), np.float32)
        srows = np.arange(n_lo, n_hi)
        sel = (srows[:, None] * L + vpos[srows]).reshape(-1)
        hwc[: (n_hi - n_lo) * LV] = h_flat[sel]
        # zero out per-seq padding columns (j >= vcnt[s])
        padm = (np.arange(LV)[None, :] >= vcnt[srows][:, None]).reshape(-1)
        hwc[: (n_hi - n_lo) * LV][padm] = 0.0
        hwcT_bf = np.ascontiguousarray(hwc.astype(bfloat16).T)

        blkid = (qid - n_lo) // BS
        cnt = np.zeros(NBLK, np.int64)
        slot = np.zeros(MC, np.int64)
        for i in range(MC):
            b = blkid[i]
            slot[i] = b * CAP + cnt[b]
            cnt[b] += 1
        slot_maps.append(slot)

        qsi_h = np.zeros(SLOT_PAD, np.int64)
        qdi_h = np.zeros(SLOT_PAD, np.int64)
        qsi_h[slot] = (qid - n_lo) * L + qsrc
        qdi_h[slot] = (qid - n_lo) * L + qdst

        # compacted pos->group: row s_local, LV entries (pad -> p2g[s, 0]).
        # remap to z_all's flat layout: shard d, local group gl lives at
        # flat element d*GC + (gl%128)*(GC//128) + gl//128
        p2g_pad = np.zeros((W, LV), np.int64)
        p2g_pad[: n_hi - n_lo] = pos2grp[srows[:, None], vpos[srows]]
        gl = p2g_pad % GC
        p2g_pad = (p2g_pad // GC) * GC + (gl % 128) * (GC // 128) + gl // 128

        # rank-8 mask factors: mask[s, p] = sum_o oh[o, s] * wm[o, p]
        # oh: one-hot of each real slot's seq offset (pad slots all-zero ->
        # mask 0 everywhere -> finite softmax of garbage, discarded on host)
        o = (qid - n_lo) % BS
        oh = np.zeros((8, NBLK * CAP), np.float32)
        for i in range(MC):
            oh[o[i], slot[i]] = 1.0
        wm = np.full((8, NBLK * LB), -240.0, np.float32)
        for b in range(NBLK):
            for oo in range(BS):
                sq = n_lo + b * BS + oo
                if sq < N_SEQ:
                    wm[oo, b * LB + oo * LV: b * LB + oo * LV + int(vcnt[sq])] = 0.0

        per_core.append({
            "hwin": hw_bf, "hwinT": hwcT_bf, "tokh": tok_hi,
            "cmat": np.ascontiguousarray(
                np.vstack([C[d * GC:(d + 1) * GC].T,
                           np.zeros((NT_PAD - N_TOK, GC), np.float32)])
                .astype(ml_dtypes.float8_e4m3).reshape(KT4, 4, 128, GC)
                .transpose(2, 0, 1, 3).reshape(128, KT4 * 4 * GC)),
            "wb16": wb16_h, "wb32": wb32_h,
            "ib16": np.ascontiguousarray(np.concatenate(
                [_wrap16(qsi_h), _wrap16(qdi_h), _wrap16(p2g_pad.reshape(-1))],
                axis=1)),
            "ohm": oh.astype(ml_dtypes.bfloat16),
            "wmm": wm.astype(ml_dtypes.float8_e4m3),
        })
    return per_core, slot_maps, (W, NBLK, BS, CAP, SLOT_PAD, LV)


def kernel(**inputs) -> np.ndarray:
    from concourse.bass_utils import run_bass_kernel_spmd

    per_core, slot_maps, key = _prep(**{k: inputs[k] for k in (
        "mem", "grp", "pos2grp", "h_grp", "msk", "idx", "src", "dst", "typ",
        "tok_emb", "Wq", "bq", "Wk", "bk", "Wrel", "brel")})
    if key not in _cache:
        _cache[key] = _build(*key)
    nc = _cache[key]
    res = run_bass_kernel_spmd(nc, per_core, core_ids=list(range(NC)))
    globals()["LAST_RESULT"] = res
    globals()["LAST_EXEC_NS"] = res.exec_time_ns
    out = np.empty((M, N_TYP), np.float32)
    for d in range(NC):
        out[d * MC:(d + 1) * MC] = res.results[d]["logitT"][:, slot_maps[d]].T
    return out


# revision 28
# speedup vs baseline: 1.3446x; 1.0550x over previous
"""Trainium2 Bass kernel for nn_AttentionModel (gnn_message_passing).

Distribution (8 cores):
  - Queries (M=8192) sharded into 8 contiguous chunks of 1024. idx is sorted,
    so each core's queries live in a contiguous window of sequences; the core
    receives h_grp for just that window (row-major bf16 for gathers +
    transposed bf16 for matmuls).
  - segment_sum z: sharded by group. Each core computes z rows [512d, 512d+512)
    as a dense count-matrix matmul  z_d = C_d @ tok_emb  (both bf16; max count
    is tiny so C is exact, tok bf16 rounding is well inside the error budget),
    then AllGather (bf16, Shared output).
  - Attention is block-diagonal: queries of one sequence attend to its own 64
    positions. Blocks of BS=8 sequences; per-block query slots padded to a
    uniform CAP so the SPMD program is static.
  - l-compaction: only positions with msk=1 participate in scores/ctx (the
    reference -inf's the rest), so the l axis is compacted host-side to LV
    valid slots per sequence (LV = max valid count, rounded to 16). hwinT
    columns, the window mask, and the z-gather list shrink by L/LV.
  - All matmuls run in bf16 (1 cyc/row on PE vs 4 for fp32); f32 accumulation
    in PSUM throughout.
  - Gathers are single-shot dma_gather (int16 indices, 16-partition wrap).
    The q gathers use transpose=True, which lands rows directly in k-major
    (dh, slot) layout — no PE transposes needed for the q path.
  - Schedule: cmat stream gets the DMA rings first (q-gathers dep on its last
    chunk), so z_my lands ~30us and the AllGather overlaps the q-gathers; the
    Pool engine then spends its serial desc-gen budget on z-gathers only.
"""

import numpy as np

N_SEQ, L, DH, DX, M, G, N_TOK, N_MEM, N_TYP = 1024, 64, 256, 128, 8192, 4096, 10000, 262144, 64
NC = 8
MC = M // NC            # queries per core
GC = G // NC            # z-groups per core
NT_PAD = ((N_TOK + 511) // 512) * 512   # 10240
KT = NT_PAD // 128
KT4 = KT // 4           # 4-k-tile DMA batches
SCALE = 1.0 / np.sqrt(np.float32(DH))

_cache = {}


def _build(W, NBLK, BS, CAP, SLOT_PAD, LV):
    import concourse.bacc as bacc
    import concourse.bass as bass
    import concourse.mybir as mybir
    import concourse.tile as tile
    from concourse.masks import make_identity
    from bass_rust import add_dep_helper

    f32 = mybir.dt.float32
    i16 = mybir.dt.int16
    bf16 = mybir.dt.bfloat16
    f8 = mybir.dt.float8e4
    LB = BS * LV                     # compacted l-columns per block (384)
    NLT = LB // 128                  # l-chunks per block (3)
    WL = W * L                       # rows of hwin (q gathers index full L)
    ZGN = NBLK * LB                  # z-gather rows (6912)
    SB = 3                           # h superblock (NBLK % 3 == 0)
    NSB = NBLK // SB
    LOOK = NBLK                      # ctx after all scores (PE FIFO: ctx waits on
                                     # gathers must not block later score matmuls)
    NCH = SLOT_PAD // 512            # qk/lq 512-slot chunks
    # ucode SWDGE desc ring holds 1024 descriptors per direction; transpose
    # gathers cost 2 rx-descs per index (512B rows), plain gathers 1 per side.
    QGW = 512                        # idxs per transposed q-gather call
    NQC = SLOT_PAD // QGW
    ZGW = 1024                       # max idxs per z-gather call
    NI = SLOT_PAD // 16 * 2 + ZGN // 16   # int16 index blob columns

    KB_CH = 5                        # kb-batches per cmat stream DMA (4 fat chunks:
                                     # DMA rings cost ~155ns/partition-line, so
                                     # fewer+fatter DMAs everywhere on the hot path)
    nc = bacc.Bacc("TRN2", target_bir_lowering=False, num_swdge_queues=1)

    hwin = nc.declare_dram_parameter("hwin", [WL, DH], bf16, isOutput=False)
    hwinT = nc.declare_dram_parameter("hwinT", [DH, W * LV], bf16, isOutput=False)
    # per-partition-contiguous tilings (one big descriptor per partition line)
    tokh = nc.declare_dram_parameter("tokh", [128, KT4 * 4 * DX], bf16, isOutput=False)
    cmat = nc.declare_dram_parameter("cmat", [128, KT4 * 4 * GC], f8, isOutput=False)
    # wb16: per-row [wqT[p], wqT[128+p], wkT[p], wkT[128+p], wrel 5x64]
    wb16 = nc.declare_dram_parameter("wb16", [128, 1858], bf16, isOutput=False)
    # wb32: [bq (2 cols), brel (rows 0:64 of col 2)]
    wb32 = nc.declare_dram_parameter("wb32", [128, 3], f32, isOutput=False)
    ib16 = nc.declare_dram_parameter("ib16", [128, NI], i16, isOutput=False)
    ohm = nc.declare_dram_parameter("ohm", [8, NBLK * CAP], bf16, isOutput=False)
    wmm = nc.declare_dram_parameter("wmm", [8, NBLK * LB], f8, isOutput=False)
    logitT = nc.declare_dram_parameter("logitT", [N_TYP, SLOT_PAD], f32, isOutput=True)

    # z_my flat [128, GC//128*DX]: local group gl lives at row gl%128, chunk
    # gl//128 (the host remaps zgi accordingly; saves a 4x descriptor fan-out)
    z_my = nc.dram_tensor("z_my", [128, GC // 128 * DX], bf16)
    z_all = nc.dram_tensor("z_all", [G, DX], bf16, addr_space="Shared")

    with tile.TileContext(nc) as tc:
        with (
            tc.tile_pool(name="const", bufs=1) as const,
            tc.tile_pool(name="persist", bufs=1) as persist,
            tc.tile_pool(name="zstream", bufs=4) as zstream,
            tc.tile_pool(name="soft", bufs=3) as soft,
        ):
            # dummy gather: forces the Q7 SWDGE ucode LOAD_LIB to start at
            # t~=1us instead of when the first real gather issues (~19us load)
            warm_idx = const.tile([128, 8], i16, tag="warmidx")
            nc.vector.memset(warm_idx[:], 0)
            warm_out = const.tile([128, 1, DH], bf16, tag="warmout")
            nc.gpsimd.dma_gather(
                out_ap=warm_out[:], in_ap=hwin.ap(), idxs_ap=warm_idx[:],
                num_idxs=128, num_idxs_reg=128, elem_size=DH, transpose=False,
            )
            ident0 = const.tile([128, 128], f32)
            make_identity(nc, ident0[:])
            # DVE-homed bf16 identity: PE transposes depend on one engine sem.
            ident = const.tile([128, 128], bf16, tag="identW")
            nc.vector.tensor_copy(ident[:], ident0[:])

            # ---- weights / small inputs (batched into few fat DMAs) ----
            wb16_sb = persist.tile([128, 1858], bf16, tag="wb16")
            nc.scalar.dma_start(wb16_sb[:], wb16[:])
            wb32_sb = persist.tile([128, 3], f32, tag="wb32")
            nc.scalar.dma_start(wb32_sb[:], wb32[:])
            ib16_sb = persist.tile([128, NI], i16, tag="ib16")
            nc.scalar.dma_start(ib16_sb[:], ib16[:])
            ohm_sb = persist.tile([8, NBLK * CAP], bf16, tag="ohm")
            nc.scalar.dma_start(ohm_sb[:], ohm[:])
            wmm_sb = persist.tile([8, NBLK * LB], f8, tag="wmm")
            nc.scalar.dma_start(wmm_sb[:], wmm[:])
            def wqT_s(b, lo, hi):
                return wb16_sb[:, b * 512 + lo:b * 512 + hi]

            def wkT_s(b, lo, hi):
                return wb16_sb[:, 1024 + b * 256 + lo:1024 + b * 256 + hi]

            def wrel_s(k):
                return wb16_sb[:, 1536 + k * 64:1536 + (k + 1) * 64]

            # front PSUM pools (Z + wqk + QK + LQ coexist): 1+1+2+2+2 = 8 banks
            zps_cm = tc.tile_pool(name="zps", bufs=1, space="PSUM"); zps = zps_cm.__enter__()
            ztps_cm = tc.tile_pool(name="ztps", bufs=1, space="PSUM"); ztps = ztps_cm.__enter__()
            qkps_cm = tc.tile_pool(name="qkps", bufs=2, space="PSUM"); qkps = qkps_cm.__enter__()
            lqps_cm = tc.tile_pool(name="lqps", bufs=2, space="PSUM"); lqps = lqps_cm.__enter__()
            pw_cm = tc.tile_pool(name="pw", bufs=1, space="PSUM"); pw = pw_cm.__enter__()

            # ---- phase Z: z_d = C_d @ tok_emb (bf16), transpose, AllGather ----
            # (Z runs FIRST on the PE; wqk after, while the AllGather flies)
            zdT = persist.tile([DX, GC], bf16, tag="zdT")
            zrow = persist.tile([128, GC // 128 * DX], bf16, tag="zrow")
            zpsum = zps.tile([DX, GC], f32)
            # tok halves + 4 fat cmat chunks interleaved on both HWDGE queues
            tokf = persist.tile([128, KT4, 4, DX], bf16, tag="tokf")
            tokh_r = tokh.rearrange("p (h r) -> h p r", h=2)
            cmat_r = cmat.rearrange("p (cc r) -> cc p r", cc=KT4 // KB_CH)
            nc.sync.dma_start(
                tokf[:, :KT4 // 2, :, :].rearrange("p a b c -> p (a b c)"), tokh_r[0])
            ck_dmas = []
            cks = []
            for cc in range(KT4 // KB_CH):
                ck = zstream.tile([128, KB_CH, 4, GC], f8, tag="ck")
                cks.append(ck)
                eng = nc.sync if cc % 2 == 0 else nc.scalar
                ck_dmas.append(eng.dma_start(
                    ck[:].rearrange("p a b c -> p (a b c)"), cmat_r[cc]))
                if cc == 0:
                    nc.sync.dma_start(
                        tokf[:, KT4 // 2:, :, :].rearrange("p a b c -> p (a b c)"),
                        tokh_r[1])
            last_z_mm = None
            for cc in range(KT4 // KB_CH):
                for kk in range(KB_CH):
                    kb = cc * KB_CH + kk
                    for j in range(4):
                        last_z_mm = nc.tensor.matmul(
                            zpsum[:], lhsT=tokf[:, kb, j, :],
                            rhs=cks[cc][:, kk, j, :],
                            start=(kb == 0 and j == 0),
                            stop=(kb == KT4 - 1 and j == 3))
            nc.vector.tensor_copy(zdT[:], zpsum[:])
            ptz = ztps.tile([128, GC // 128, 128], bf16, tag="ztp")
            for c in range(GC // 128):
                nc.tensor.transpose(ptz[:, c, :], zdT[:, c * 128:(c + 1) * 128], ident[:])
            nc.vector.tensor_copy(zrow[:], ptz[:])
            zmy_dma = nc.sync.dma_start(z_my[:], zrow[:])
            ag_inst = nc.gpsimd.collective_compute(
                "AllGather", mybir.AluOpType.bypass,
                replica_groups=[list(range(NC))],
                ins=[z_my.ap().opt()], outs=[z_all.ap().opt()],
            )

            # Wqk = Wq @ Wk^T (bf16 in, f32 acc); bqk = Wk^T^T @ bq
            wqk_sb = [persist.tile([128, DH], bf16, tag=f"wqk{a}", name=f"wqk{a}") for a in range(4)]
            bqk_sb = [persist.tile([128, 1], f32, tag=f"bqk{c}", name=f"bqk{c}") for c in range(2)]
            for a in range(4):
                pwt = pw.tile([128, DH], f32, tag="wqkps")
                for b in range(2):
                    nc.tensor.matmul(pwt[:], lhsT=wqT_s(b, a * 128, (a + 1) * 128),
                                     rhs=wkT_s(b, 0, 256), start=(b == 0), stop=(b == 1))
                nc.vector.tensor_copy(wqk_sb[a][:], pwt[:])
            for c in range(2):
                pb = pw.tile([128, 1], f32, tag="bqkps")
                for b in range(2):
                    nc.tensor.matmul(pb[:], lhsT=wkT_s(b, c * 128, (c + 1) * 128),
                                     rhs=wb16_sb[:, 1856 + b:1857 + b],
                                     start=(b == 0), stop=(b == 1))
                nc.vector.tensor_copy(bqk_sb[c][:], pb[:])

            # ---- hT whole-window loads (one fat DMA per dh half) ----
            hTw = [persist.tile([128, W * LV], bf16, tag=f"hTw{c}", name=f"hTw{c}")
                   for c in range(2)]
            for c in range(2):
                h_dma = nc.scalar.dma_start(hTw[c][:], hwinT[c * 128:(c + 1) * 128, :])
                add_dep_helper(h_dma.ins, ck_dmas[3 - c].ins,
                               reason="h loads after cmat stream")

            # ---- q gathers: transposed single-shot -> k-major qT tiles ----
            # qT layout: chunk-major [128, NQC, 2, QGW]; dh dim j*128+d of
            # slot qc*QGW+s at [d, qc, j, s]
            qgT = [persist.tile([128, NQC, 2, QGW], bf16, tag=f"qgT{h}", name=f"qgT{h}")
                   for h in range(2)]
            for h in range(2):
                ib_off = h * (SLOT_PAD // 16)
                for qc in range(NQC):
                    qgi = nc.gpsimd.dma_gather(
                        out_ap=qgT[h][:, qc, :, :],
                        in_ap=hwin.ap(),
                        idxs_ap=ib16_sb[:, ib_off + qc * (QGW // 16):
                                        ib_off + (qc + 1) * (QGW // 16)],
                        num_idxs=QGW, num_idxs_reg=QGW, elem_size=DH,
                        transpose=True,
                    )
                    if h == 0 and qc == 0:
                        # keep gather ring traffic off the cmat stream: the
                        # last Z matmul implies every cmat chunk has landed
                        add_dep_helper(qgi.ins, last_z_mm.ins,
                                       reason="q gathers after cmat stream")

            def qt_a(a, ch):
                return qgT[a // 2][:, ch, a % 2, :]

            # ---- phase QK/LQ per 512-slot chunk ----
            qkT = [persist.tile([128, SLOT_PAD], bf16, tag=f"qkT{c}", name=f"qkT{c}") for c in range(2)]
            logit_q = persist.tile([N_TYP, SLOT_PAD], f32, tag="logit_q")
            for ch in range(NCH):
                sl = slice(ch * 512, (ch + 1) * 512)
                for c in range(2):
                    pq = qkps.tile([128, 512], f32, tag="qkp")
                    for a in range(4):
                        nc.tensor.matmul(pq[:], lhsT=wqk_sb[a][:, c * 128:(c + 1) * 128],
                                         rhs=qt_a(a, ch), start=(a == 0), stop=(a == 3))
                    nc.scalar.activation(qkT[c][:, sl], pq[:],
                                         mybir.ActivationFunctionType.Identity,
                                         bias=bqk_sb[c][:, :1])
                pl = lqps.tile([N_TYP, 512], f32, tag="lqp")
                for a in range(4):
                    nc.tensor.matmul(pl[:], lhsT=wrel_s(a), rhs=qt_a(a, ch),
                                     start=(a == 0), stop=(a == 3))
                nc.scalar.activation(logit_q[:, sl], pl[:],
                                     mybir.ActivationFunctionType.Identity,
                                     bias=wb32_sb[0:N_TYP, 2:3])

            pw_cm.__exit__(None, None, None)
            lqps_cm.__exit__(None, None, None)
            qkps_cm.__exit__(None, None, None)
            ztps_cm.__exit__(None, None, None)
            zps_cm.__exit__(None, None, None)

            # ---- z gathers: chunked single-shot dma_gather (after AG) ----
            zg_all = persist.tile([128, NBLK * NLT, DX], bf16, tag="zg_all")
            zoff = 0
            while zoff < ZGN:
                cnt = min(ZGW, ZGN - zoff)
                nc.gpsimd.dma_gather(
                    out_ap=zg_all[:, zoff // 128:(zoff + cnt) // 128, :],
                    in_ap=z_all.ap(),
                    idxs_ap=ib16_sb[:, SLOT_PAD // 8 + zoff // 16:
                                    SLOT_PAD // 8 + (zoff + cnt) // 16],
                    num_idxs=cnt, num_idxs_reg=cnt, elem_size=DX,
                    transpose=False,
                )
                zoff += cnt

            # ---- phase S: scores/softmax/attnT (S1) + ctx (S2), interleaved ----
            ctxT = persist.tile([128, SLOT_PAD], bf16, tag="ctxT")
            if NBLK * CAP < SLOT_PAD:
                nc.vector.memset(ctxT[:, NBLK * CAP:], 0.0)
            with (
                tc.tile_pool(name="sps", bufs=2, space="PSUM") as sps,
                tc.tile_pool(name="atps", bufs=3, space="PSUM") as atps,
                tc.tile_pool(name="cps", bufs=3, space="PSUM") as cps,
            ):
                am = None
                aT = {}
                for bb in range(NBLK + LOOK):
                    if bb < NBLK:
                        b = bb
                        hT = [hTw[c][:, b * LB:(b + 1) * LB] for c in range(2)]

                        ps_s = sps.tile([CAP, LB], f32, tag="sps")
                        for c in range(2):
                            nc.tensor.matmul(ps_s[:], lhsT=qkT[c][:, b * CAP:b * CAP + CAP],
                                             rhs=hT[c], start=(c == 0), stop=False)
                        # mask is rank-8: one-hot(slot seq-offset) x window-mask rows
                        nc.tensor.matmul(ps_s[:], lhsT=ohm_sb[:, b * CAP:b * CAP + CAP],
                                         rhs=wmm_sb[:, b * LB:(b + 1) * LB],
                                         start=False, stop=True)
                        e = soft.tile([CAP, LB], bf16, tag="e", bufs=2)
                        den = soft.tile([CAP, 1], f32, tag="den")
                        nc.scalar.activation(e[:], ps_s[:], mybir.ActivationFunctionType.Exp,
                                             scale=float(SCALE), accum_out=den[:])
                        rec = soft.tile([CAP, 1], f32, tag="rec")
                        nc.vector.reciprocal(rec[:], den[:])
                        attn = soft.tile([CAP, LB], bf16, tag="attn")
                        nc.vector.tensor_scalar_mul(attn[:], e[:], rec[:])

                        pta = atps.tile([128, NLT, CAP], bf16, tag="atp")
                        for k in range(NLT):
                            nc.tensor.transpose(pta[:, k, :], attn[:, k * 128:(k + 1) * 128],
                                                ident[:CAP, :CAP])
                        aT[b] = soft.tile([128, NLT * CAP], bf16, tag="aT", bufs=LOOK + 2,
                                          name=f"aT{b}")
                        nc.vector.tensor_copy(aT[b][:], pta[:])
                    if bb >= LOOK:
                        b2 = bb - LOOK
                        ps_c = cps.tile([DX, CAP], f32, tag="cps")
                        for k in range(NLT):
                            nc.tensor.matmul(ps_c[:], lhsT=zg_all[:, b2 * NLT + k, :],
                                             rhs=aT[b2][:, k * CAP:(k + 1) * CAP],
                                             start=(k == 0), stop=(k == NLT - 1))
                        nc.scalar.activation(ctxT[:, b2 * CAP:b2 * CAP + CAP], ps_c[:],
                                             mybir.ActivationFunctionType.Copy)
                        del aT[b2]

            # ---- phase L: logitT = logit_q + WrelC^T @ ctxT ----
            with tc.tile_pool(name="lps", bufs=2, space="PSUM") as lps:
                for ch in range(NCH):
                    w = min(512, SLOT_PAD - ch * 512)
                    sl = slice(ch * 512, ch * 512 + w)
                    pl = lps.tile([N_TYP, 512], f32, tag="lps")
                    nc.tensor.matmul(pl[:, :w], lhsT=wrel_s(4),
                                     rhs=ctxT[:, sl],
                                     start=True, stop=True)
                    lg = soft.tile([N_TYP, 512], f32, tag="lg", bufs=2)
                    nc.vector.tensor_add(lg[:, :w], pl[:, :w], logit_q[:, sl])
                    nc.scalar.dma_start(logitT[:, sl], lg[:, :w])

    nc.compile()
    return nc


def _wrap16(flat):
    """int16 gather-index layout: index i at [i % 16, i // 16], rows tiled to 128."""
    a = np.asarray(flat, np.int16).reshape(-1, 16).T
    return np.ascontiguousarray(np.tile(a, (8, 1)))


def _prep(mem, grp, pos2grp, h_grp, msk, idx, src, dst, typ, tok_emb, Wq, bq, Wk, bk, Wrel, brel):
    """Host-side sharding/layout. Integer index work + relayout only."""
    import ml_dtypes
    bfloat16 = ml_dtypes.bfloat16
    idx = np.asarray(idx, np.int64)
    src = np.asarray(src, np.int64)
    dst = np.asarray(dst, np.int64)
    mem = np.asarray(mem, np.int64)
    grp = np.asarray(grp, np.int64)
    pos2grp = np.asarray(pos2grp, np.int64)
    msk = np.asarray(msk)
    h_grp = np.asarray(h_grp, np.float32)
    tok_emb = np.asarray(tok_emb, np.float32)

    # ---- count matrix for segment_sum ----
    C = np.bincount(grp * N_TOK + mem, minlength=G * N_TOK).reshape(G, N_TOK).astype(np.float32)

    # ---- per-core windows ----
    starts = np.array([idx[d * MC] for d in range(NC)])
    ends = np.array([idx[(d + 1) * MC - 1] for d in range(NC)])
    BS = 8
    Wmax = int((ends - starts).max()) + 1
    W = -(-Wmax // (3 * BS)) * (3 * BS)

    maxc = 0
    for d in range(NC):
        blkid = (idx[d * MC:(d + 1) * MC] - starts[d]) // BS
        maxc = max(maxc, int(np.bincount(blkid).max()))
    if maxc > 128:
        BS = 4
        W = -(-Wmax // (3 * BS)) * (3 * BS)
        maxc = 0
        for d in range(NC):
            blkid = (idx[d * MC:(d + 1) * MC] - starts[d]) // BS
            maxc = max(maxc, int(np.bincount(blkid).max()))
        assert maxc <= 128, f"block occupancy {maxc} > 128 even at BS=4"
    CAP = -(-maxc // 32) * 32
    NBLK = W // BS
    SLOT_PAD = -(-(NBLK * CAP) // 512) * 512
    # l-compaction: LV = max valid positions per sequence, 16-aligned so
    # BS*LV is a multiple of 128 (BS=8).
    if BS == 8:
        lv_max = int(msk.sum(axis=1).max())
        LV = min(L, -(-lv_max // 16) * 16)
    else:
        LV = L
    LB = BS * LV

    # per-seq valid position lists, padded with position 0 (masked out)
    vcnt = msk.sum(axis=1).astype(np.int64)
    vpos = np.zeros((N_SEQ, LV), np.int64)
    for s in range(N_SEQ):
        v = np.flatnonzero(msk[s])[:LV]
        vpos[s, :len(v)] = v

    tok_pad = np.vstack([tok_emb, np.zeros((NT_PAD - N_TOK, DX), np.float32)])
    # per-partition-contiguous tiling: [128, KT4*4*DX], line p holds k-rows
    # {kb*512 + j*128 + p} for all (kb, j)
    tok_hi = np.ascontiguousarray(
        tok_pad.astype(bfloat16).reshape(KT4, 4, 128, DX)
        .transpose(2, 0, 1, 3).reshape(128, KT4 * 4 * DX))
    wqT_h = np.asarray(Wq, np.float32).T.astype(bfloat16)
    wkT_h = np.asarray(Wk, np.float32).T.astype(bfloat16)
    wrel_h = np.asarray(Wrel, np.float32).astype(bfloat16)
    wb16_h = np.ascontiguousarray(np.concatenate(
        [wqT_h[:128], wqT_h[128:], wkT_h[:128], wkT_h[128:]]
        + [wrel_h[k * 128:(k + 1) * 128] for k in range(5)], axis=1))
    bq_cols = np.asarray(bq, np.float32).reshape(2, 128).T
    wb16_h = np.ascontiguousarray(np.concatenate(
        [wb16_h, bq_cols.astype(bfloat16)], axis=1))
    wb32_h = np.zeros((128, 3), np.float32)
    wb32_h[:N_TYP, 2] = np.asarray(brel, np.float32)

    h_flat = np.ascontiguousarray(h_grp.reshape(N_SEQ * L, DH))
    per_core = []
    slot_maps = []
    for d in range(NC):
        n_lo = int(starts[d])
        qid = idx[d * MC:(d + 1) * MC]
        qsrc = src[d * MC:(d + 1) * MC]
        qdst = dst[d * MC:(d + 1) * MC]

        hw = np.zeros((W * L, DH), np.float32)
        n_hi = min(n_lo + W, N_SEQ)
        hw[: (n_hi - n_lo) * L] = h_flat[n_lo * L: n_hi * L]
        hw_bf = hw.astype(bfloat16)

        # compacted transposed window: column (s_local*LV + j) = h[s, vpos[s, j]]
        hwc = np.zeros((W * LV, DH), np.float32)
        srows = np.arange(n_lo, n_hi)
        sel = (srows[:, None] * L + vpos[srows]).reshape(-1)
        hwc[: (n_hi - n_lo) * LV] = h_flat[sel]
        # zero out per-seq padding columns (j >= vcnt[s])
        padm = (np.arange(LV)[None, :] >= vcnt[srows][:, None]).reshape(-1)
        hwc[: (n_hi - n_lo) * LV][padm] = 0.0
        hwcT_bf = np.ascontiguousarray(hwc.astype(bfloat16).T)

        blkid = (qid - n_lo) // BS
        cnt = np.zeros(NBLK, np.int64)
        slot = np.zeros(MC, np.int64)
        for i in range(MC):
            b = blkid[i]
            slot[i] = b * CAP + cnt[b]
            cnt[b] += 1
        slot_maps.append(slot)

        qsi_h = np.zeros(SLOT_PAD, np.int64)
        qdi_h = np.zeros(SLOT_PAD, np.int64)
        qsi_h[slot] = (qid - n_lo) * L + qsrc
        qdi_h[slot] = (qid - n_lo) * L + qdst

        # compacted pos->group: row s_local, LV entries (pad -> p2g[s, 0]).
        # remap to z_all's flat layout: shard d, local group gl lives at
        # flat element d*GC + (gl%128)*(GC//128) + gl//128
        p2g_pad = np.zeros((W, LV), np.int64)
        p2g_pad[: n_hi - n_lo] = pos2grp[srows[:, None], vpos[srows]]
        gl = p2g_pad % GC
        p2g_pad = (p2g_pad // GC) * GC + (gl % 128) * (GC // 128) + gl // 128

        # rank-8 mask factors: mask[s, p] = sum_o oh[o, s] * wm[o, p]
        # oh: one-hot of each real slot's seq offset (pad slots all-zero ->
        # mask 0 everywhere -> finite softmax of garbage, discarded on host)
        o = (qid - n_lo) % BS
        oh = np.zeros((8, NBLK * CAP), np.float32)
        for i in range(MC):
            oh[o[i], slot[i]] = 1.0
        wm = np.full((8, NBLK * LB), -240.0, np.float32)
        for b in range(NBLK):
            for oo in range(BS):
                sq = n_lo + b * BS + oo
                if sq < N_SEQ:
                    wm[oo, b * LB + oo * LV: b * LB + oo * LV + int(vcnt[sq])] = 0.0

        per_core.append({
            "hwin": hw_bf, "hwinT": hwcT_bf, "tokh": tok_hi,
            "cmat": np.ascontiguousarray(
                np.vstack([C[d * GC:(d + 1) * GC].T,
                           np.zeros((NT_PAD - N_TOK, GC), np.float32)])
                .astype(ml_dtypes.float8_e4m3).reshape(KT4, 4, 128, GC)
                .transpose(2, 0, 1, 3).reshape(128, KT4 * 4 * GC)),
            "wb16": wb16_h, "wb32": wb32_h,
            "ib16": np.ascontiguousarray(np.concatenate(
                [_wrap16(qsi_h), _wrap16(qdi_h), _wrap16(p2g_pad.reshape(-1))],
                axis=1)),
            "ohm": oh.astype(ml_dtypes.bfloat16),
            "wmm": wm.astype(ml_dtypes.float8_e4m3),
        })
    return per_core, slot_maps, (W, NBLK, BS, CAP, SLOT_PAD, LV)


def kernel(**inputs) -> np.ndarray:
    from concourse.bass_utils import run_bass_kernel_spmd

    per_core, slot_maps, key = _prep(**{k: inputs[k] for k in (
        "mem", "grp", "pos2grp", "h_grp", "msk", "idx", "src", "dst", "typ",
        "tok_emb", "Wq", "bq", "Wk", "bk", "Wrel", "brel")})
    if key not in _cache:
        _cache[key] = _build(*key)
    nc = _cache[key]
    res = run_bass_kernel_spmd(nc, per_core, core_ids=list(range(NC)))
    globals()["LAST_RESULT"] = res
    globals()["LAST_EXEC_NS"] = res.exec_time_ns
    out = np.empty((M, N_TYP), np.float32)
    for d in range(NC):
        out[d * MC:(d + 1) * MC] = res.results[d]["logitT"][:, slot_maps[d]].T
    return out
